# revision 10
# baseline (speedup 1.0000x reference)
"""FP4Net (bnb-FP4 quantize-dequantize 4-layer MLP) Trainium2 kernel.

Strategy (8 NeuronCores):
  - Data-parallel over batch for the matmuls: each core handles 1024 of 8192 rows.
  - FP4 quant-dequant of the weights is sharded 8x across cores (by output-row
    blocks, keeping the 64-elem FP4 blocks intact), computed exactly with fp32
    bit tricks on the vector engine, stored transposed (W.T layout) in fp16,
    then AllGathered so every core has all dequantized weights.
  - 4 chained fp16 matmul layers (fp32 PSUM accumulate), bias+ReLU / Sigmoid
    fused on the scalar engine, activations resident in SBUF feature-major.

Rounding trick: with g = 3*w/scale, the bnb FP4 codebook {0, 1/192, 1/6, 1/4,
1/3, 1/2, 2/3, 1} maps to {0, 1/64, 1/2, 3/4, 1, 3/2, 2, 3}: round-to-nearest
over that set == round g to 1 mantissa bit (round-half-up via integer bit add),
clamped below at 1/2, with a separate two-threshold step for the {0, 1/64}
region. Verified bit-exact vs the jax reference modulo ~1-ulp boundary fuzz
(1 flipped element per ~16M weights on the actual inputs).
"""
import sys
import numpy as np

for _p in ("/opt/trn_rl_repo", "/root/.axon_site/_ro/trn_rl_repo"):
    if _p not in sys.path:
        sys.path.append(_p)

N_CORES = 8
B, IN, H, OUT = 8192, 1024, 4096, 1024
BS = B // N_CORES          # batch shard per core
HS = H // N_CORES          # hidden-row shard per core (w1/w2/w3)
OS = OUT // N_CORES        # out-row shard per core (w4)

# FP4 codebook-derived threshold constants (g-space = 3*norm), f64 precision
_FP4_POS = np.array([0.0, 0.0052083333, 0.6666667, 1.0, 0.3333333, 0.5,
                     0.1666667, 0.25], dtype=np.float32)
_CS = np.sort(_FP4_POS).astype(np.float64)
_TL = np.float32(3.0 * (_CS[0] + _CS[1]) / 2.0)
_TH = np.float32(3.0 * (_CS[1] + _CS[2]) / 2.0)
# thresholds shifted by the +0x200000 rounding bias, compared in float domain
TLS = float(np.float32((_TL.view(np.uint32) + np.uint32(0x00200000)).view(np.float32)))
THS = float(np.float32((_TH.view(np.uint32) + np.uint32(0x00200000)).view(np.float32)))
LO_BITS = int(np.float32(1.0 / 64).view(np.uint32))   # 0x3C800000
BIG_BITS = 0x40400000                                  # bits of 3.0


def _i32(x):
    return int(np.uint32(x).view(np.int32))


_CACHED = {}


def _build_nc(taps=False):
    import concourse.bass as bass
    import concourse.mybir as mybir
    import concourse.tile as tile
    from concourse import bacc

    dt = mybir.dt
    Alu = mybir.AluOpType
    Act = mybir.ActivationFunctionType

    nc = bacc.Bacc("TRN2", target_bir_lowering=False, debug=False,
                   num_devices=N_CORES)

    # ---- I/O ----
    xs = nc.dram_tensor("xs", [BS, IN], dt.float32, kind="ExternalInput")
    w_in = {
        1: nc.dram_tensor("w1s", [HS, IN], dt.float32, kind="ExternalInput"),
        2: nc.dram_tensor("w2s", [HS, H], dt.float32, kind="ExternalInput"),
        3: nc.dram_tensor("w3s", [HS, H], dt.float32, kind="ExternalInput"),
        4: nc.dram_tensor("w4s", [OS, H], dt.float32, kind="ExternalInput"),
    }
    b_in = {
        1: nc.dram_tensor("b1", [H // 128, 128], dt.float32, kind="ExternalInput"),
        2: nc.dram_tensor("b2", [H // 128, 128], dt.float32, kind="ExternalInput"),
        3: nc.dram_tensor("b3", [H // 128, 128], dt.float32, kind="ExternalInput"),
        4: nc.dram_tensor("b4", [OUT // 128, 128], dt.float32, kind="ExternalInput"),
    }
    y_out = nc.dram_tensor("y", [OUT, BS], dt.float32, kind="ExternalOutput")

    # weight dims per layer: (rows of W == dout, cols of W == k/contraction)
    WDIMS = {1: (H, IN), 2: (H, H), 3: (H, H), 4: (OUT, H)}

    # ---- internal DRAM: dequantized W.T-layout shards + AllGather outputs ----
    # layout [n_h_tiles, K, 128]: tile j of lhsT (k-major, h-in-tile minor)
    dq_shard = {}
    dq_full = {}
    for l, (dout, k) in WDIMS.items():
        rs = dout // N_CORES  # shard rows
        dq_shard[l] = nc.dram_tensor(f"dqs{l}", [rs // 128, k, 128], dt.float16)
        dq_full[l] = nc.dram_tensor(f"dqf{l}", [dout // 128, k, 128], dt.float16,
                                    addr_space="Shared")

    tap_t = {}
    if taps:
        for l, (dout, k) in WDIMS.items():
            tap_t[f"dq{l}"] = nc.dram_tensor(f"tap_dq{l}", [dout // 128, k, 128],
                                             dt.float16, kind="ExternalOutput")
        tap_t["a0"] = nc.dram_tensor("tap_a0", [128, (IN // 128) * BS],
                                     dt.float16, kind="ExternalOutput")
        for l in (1, 2, 3):
            tap_t[f"a{l}"] = nc.dram_tensor(f"tap_a{l}", [128, (H // 128) * BS],
                                            dt.float16, kind="ExternalOutput")

    FDQ = 512          # dequant chunk free-size (fp32 elems per partition)
    NBQ = FDQ // 64    # fp4 blocks per chunk

    with tile.TileContext(nc) as tc:
        with (
            tc.tile_pool(name="const", bufs=1) as cpool,
            tc.tile_pool(name="bias", bufs=1) as bpool,
            tc.tile_pool(name="xload", bufs=2) as xpool,
            tc.tile_pool(name="a0", bufs=1) as a0pool,
            tc.tile_pool(name="acts", bufs=2) as apool,
            tc.tile_pool(name="dqin", bufs=2) as dqin_pool,
            tc.tile_pool(name="dqtmp", bufs=1) as dqtmp_pool,
            tc.tile_pool(name="dqout", bufs=2) as dqout_pool,
            tc.tile_pool(name="wt", bufs=2) as wpool,
            tc.tile_pool(name="psum", bufs=4, space="PSUM") as pspool,
        ):
            # int32 constants for scalar_tensor_tensor scalars
            c_lo = cpool.tile([128, 1], dt.int32)
            nc.vector.memset(c_lo[:], _i32(LO_BITS))
            c_half = cpool.tile([128, 1], dt.int32)
            nc.vector.memset(c_half[:], _i32(0x3F000000))
            c_sign = cpool.tile([128, 1], dt.int32)
            nc.vector.memset(c_sign[:], _i32(0x80000000))

            # ---- biases -> SBUF [128, ntiles] ----
            b_sb = {}
            for l, (dout, _k) in WDIMS.items():
                nj = dout // 128
                bt = bpool.tile([128, nj], dt.float32, tag=f"bias{l}")
                for j in range(nj):
                    nc.sync.dma_start(bt[:, j:j + 1], b_in[l][j])
                b_sb[l] = bt

            # ---- x load + cast fp16 + transpose -> A0 [128, IN/128, BS] ----
            a_cur = a0pool.tile([128, IN // 128, BS], dt.float16)
            for bt_i in range(BS // 128):
                xt = xpool.tile([128, IN], dt.float32, tag="xt")
                nc.sync.dma_start(xt[:], xs[bt_i * 128:(bt_i + 1) * 128, :])
                xh = xpool.tile([128, IN], dt.float16, tag="xh")
                nc.vector.tensor_copy(xh[:], xt[:])
                for jk in range(IN // 128):
                    nc.sync.dma_start_transpose(
                        a_cur[:, jk, bt_i * 128:(bt_i + 1) * 128],
                        xh[:, jk * 128:(jk + 1) * 128])

            # ---- dequant (sharded) + transpose-store + AllGather ----
            for l, (dout, K) in WDIMS.items():
                rs = dout // N_CORES
                for r in range(rs // 128):
                    for cix in range(K // FDQ):
                        w = dqin_pool.tile([128, NBQ, 64], dt.float32, tag="dqw")
                        nc.sync.dma_start(
                            w[:],
                            w_in[l][r * 128:(r + 1) * 128,
                                    cix * FDQ:(cix + 1) * FDQ]
                            .rearrange("p (b i) -> p b i", i=64))
                        scale = dqtmp_pool.tile([128, NBQ, 1], dt.float32, tag="scale")
                        nc.vector.tensor_reduce(scale[:], w[:],
                                                axis=mybir.AxisListType.X,
                                                op=Alu.max,
                                                apply_absolute_value=True)
                        recip = dqtmp_pool.tile([128, NBQ, 1], dt.float32, tag="recip")
                        nc.vector.reciprocal(recip[:], scale[:])
                        s3 = dqtmp_pool.tile([128, NBQ, 1], dt.float32, tag="s3")
                        nc.vector.tensor_scalar_mul(s3[:], scale[:], 1.0 / 3.0)
                        g = dqtmp_pool.tile([128, NBQ, 64], dt.float32, tag="g")
                        nc.vector.scalar_tensor_tensor(
                            g[:], w[:], 3.0, recip[:].broadcast_to((128, NBQ, 64)),
                            op0=Alu.mult, op1=Alu.mult)
                        gi = g[:].bitcast(dt.int32)
                        # NOTE: DVE ops must never write in-place onto their
                        # own input (dual-port perf modes race), and int adds
                        # must keep few significant bits (the int ALU path is
                        # fp32-internal, so ~2^30-magnitude adds round).
                        ta = dqtmp_pool.tile([128, NBQ, 64], dt.int32, tag="ta")
                        nc.vector.tensor_scalar(ta[:], gi, _i32(0x7FFFFFFF), None,
                                                op0=Alu.bitwise_and)  # m0 = |g| bits
                        tb = dqtmp_pool.tile([128, NBQ, 64], dt.int32, tag="tb")
                        nc.vector.tensor_scalar(tb[:], ta[:], _i32(0xFFC00000), None,
                                                op0=Alu.bitwise_and)  # trunc
                        tc_ = dqtmp_pool.tile([128, NBQ, 64], dt.int32, tag="tc")
                        nc.vector.tensor_scalar(tc_[:], ta[:], _i32(0x00200000),
                                                _i32(1), op0=Alu.bitwise_and,
                                                op1=Alu.logical_shift_left)  # half-bit<<1
                        te = dqtmp_pool.tile([128, NBQ, 64], dt.int32, tag="te")
                        nc.vector.tensor_tensor(te[:], tb[:], tc_[:],
                                                op=Alu.add)  # r2a (exact: 10+1 sig bits)
                        af = ta[:].bitcast(dt.float32)  # |g| as float
                        # M1/M2 masks: all-ones iff |g| > TL / TH
                        nc.vector.tensor_scalar(tb[:], af, float(_TL), 1.0,
                                                op0=Alu.is_le, op1=Alu.subtract)
                        td = dqtmp_pool.tile([128, NBQ, 64], dt.int32, tag="td")
                        nc.vector.tensor_scalar(td[:], af, float(_TH), 1.0,
                                                op0=Alu.is_le, op1=Alu.subtract)
                        # S2 = M2 & BIG -> ta (m0 dead)
                        nc.vector.tensor_scalar(ta[:], td[:], _i32(BIG_BITS), None,
                                                op0=Alu.bitwise_and)
                        # sel = (M1 & LO) | S2  -> tc_
                        nc.vector.scalar_tensor_tensor(
                            tc_[:], tb[:], c_lo[:], ta[:],
                            op0=Alu.bitwise_and, op1=Alu.bitwise_or)
                        # mag = min(max(r2a, 0.5), sel) -> tb
                        nc.vector.scalar_tensor_tensor(
                            tb[:], te[:], c_half[:], tc_[:],
                            op0=Alu.max, op1=Alu.min)
                        # dqb = signbit(g) | mag -> ta
                        nc.vector.scalar_tensor_tensor(
                            ta[:], gi, c_sign[:], tb[:],
                            op0=Alu.bitwise_and, op1=Alu.bitwise_or)
                        dq = dqout_pool.tile([128, NBQ, 64], dt.float16, tag="dq")
                        nc.vector.tensor_tensor(
                            dq[:], ta[:].bitcast(dt.float32),
                            s3[:].broadcast_to((128, NBQ, 64)), op=Alu.mult)
                        # transpose-store: [128 h, FDQ k] -> W.T layout shard
                        dqt = dqout_pool.tile([128, FDQ // 128, 128], dt.float16,
                                              tag="dqt")
                        dqf = dq[:].rearrange("p b i -> p (b i)")
                        for ck in range(FDQ // 128):
                            nc.sync.dma_start_transpose(
                                dqt[:, ck, :], dqf[:, ck * 128:(ck + 1) * 128])
                        nc.sync.dma_start(
                            dq_shard[l][r, cix * FDQ:(cix + 1) * FDQ, :]
                            .rearrange("(c p) h -> p c h", p=128),
                            dqt[:])
                nc.gpsimd.collective_compute(
                    "AllGather", Alu.bypass,
                    replica_groups=[list(range(N_CORES))],
                    ins=[dq_shard[l][:]],
                    outs=[dq_full[l][:]],
                )
                if taps:
                    nc.sync.dma_start(tap_t[f"dq{l}"][:], dq_full[l][:])

            if taps:
                nc.sync.dma_start(tap_t["a0"][:],
                                  a_cur[:].rearrange("p j b -> p (j b)"))

            # ---- matmul layers ----
            for l, (dout, K) in WDIMS.items():
                nj = dout // 128
                nk = K // 128
                out_dt = dt.float32 if l == 4 else dt.float16
                a_next = apool.tile([128, nj, BS], out_dt, tag="acts")
                func = Act.Sigmoid if l == 4 else Act.Relu
                for j in range(nj):
                    wt = wpool.tile([128, nk, 128], dt.float16, tag="wt")
                    for i in range(nk):
                        nc.sync.dma_start(
                            wt[:, i, :],
                            dq_full[l][j, i * 128:(i + 1) * 128, :])
                    ps = []
                    for _n in range(BS // 512):
                        ps_t = pspool.tile([128, 512], dt.float32, tag="ps")
                        ps.append(ps_t)
                    for i in range(nk):
                        for n in range(BS // 512):
                            nc.tensor.matmul(
                                ps[n][:], wt[:, i, :],
                                a_cur[:, i, n * 512:(n + 1) * 512],
                                start=(i == 0), stop=(i == nk - 1))
                    for n in range(BS // 512):
                        nc.scalar.activation(
                            a_next[:, j, n * 512:(n + 1) * 512], ps[n][:],
                            func, bias=b_sb[l][:, j:j + 1], scale=1.0)
                if taps and l < 4:
                    nc.sync.dma_start(tap_t[f"a{l}"][:],
                                      a_next[:].rearrange("p j b -> p (j b)"))
                a_cur = a_next

            # ---- output: feature-major [OUT, BS] ----
            for j in range(OUT // 128):
                nc.sync.dma_start(y_out[j * 128:(j + 1) * 128, :], a_cur[:, j, :])

    nc.compile()
    return nc


def _get_nc():
    if "nc" not in _CACHED:
        _CACHED["nc"] = _build_nc()
    return _CACHED["nc"]


def kernel(**inputs):
    from concourse.bass_utils import run_bass_kernel_spmd

    x = np.ascontiguousarray(np.asarray(inputs["x"], dtype=np.float32))
    ws = {l: np.ascontiguousarray(np.asarray(inputs[f"w{l}"], dtype=np.float32))
          for l in (1, 2, 3, 4)}
    bs = {l: np.ascontiguousarray(
        np.asarray(inputs[f"b{l}"], dtype=np.float32).reshape(-1, 128))
        for l in (1, 2, 3, 4)}

    nc = _get_nc()
    in_maps = []
    for c in range(N_CORES):
        m = {
            "xs": x[c * BS:(c + 1) * BS],
            "w1s": ws[1][c * HS:(c + 1) * HS],
            "w2s": ws[2][c * HS:(c + 1) * HS],
            "w3s": ws[3][c * HS:(c + 1) * HS],
            "w4s": ws[4][c * OS:(c + 1) * OS],
            "b1": bs[1], "b2": bs[2], "b3": bs[3], "b4": bs[4],
        }
        in_maps.append(m)

    res = run_bass_kernel_spmd(nc, in_maps, list(range(N_CORES)))
    out = np.empty((B, OUT), dtype=np.float32)
    for c in range(N_CORES):
        out[c * BS:(c + 1) * BS] = res.results[c]["y"].T
    return out


if __name__ == "__main__":
    rng = np.random.default_rng(0)
    ins = {
        "x": rng.standard_normal((B, IN)).astype(np.float32),
        "w1": (rng.standard_normal((H, IN)) * 0.1).astype(np.float32),
        "b1": np.zeros(H, np.float32),
        "w2": (rng.standard_normal((H, H)) * 0.1).astype(np.float32),
        "b2": np.zeros(H, np.float32),
        "w3": (rng.standard_normal((H, H)) * 0.1).astype(np.float32),
        "b3": np.zeros(H, np.float32),
        "w4": (rng.standard_normal((OUT, H)) * 0.1).astype(np.float32),
        "b4": np.zeros(OUT, np.float32),
    }
    y = kernel(**ins)
    print("kernel ran, output shape", y.shape, "mean", float(y.mean()))


# revision 12
# speedup vs baseline: 1.3225x; 1.3225x over previous
"""FP4Net (bnb-FP4 quantize-dequantize 4-layer MLP) Trainium2 kernel.

Strategy (8 NeuronCores):
  - Data-parallel over batch for the matmuls: each core handles 1024 of 8192 rows.
  - FP4 quant-dequant of the weights is sharded 8x across cores (by output-row
    blocks, keeping the 64-elem FP4 blocks intact), computed exactly with fp32
    bit tricks on the vector engine, stored transposed (W.T layout) in fp16,
    then AllGathered so every core has all dequantized weights.
  - 4 chained fp16 matmul layers (fp32 PSUM accumulate); bias+ReLU epilogues on
    the vector engine, sigmoid on the scalar engine; activations resident in
    SBUF feature-major.
  - Engine streams are kept separate to avoid sequencer head-of-line blocking:
    SP: bias loads + weight-strip loads + output stores;
    ACT: x staging + all dequant-phase DMAs + final sigmoid;
    DVE: dequant compute + ReLU epilogues (emission-interleaved);
    GpSimd: AllGathers. Dequant of weight l+1 overlaps layer l's matmuls.

Rounding trick: with g = 3*w/scale, the bnb FP4 codebook {0, 1/192, 1/6, 1/4,
1/3, 1/2, 2/3, 1} maps to {0, 1/64, 1/2, 3/4, 1, 3/2, 2, 3}: round-to-nearest
over that set == round g to 1 stored mantissa bit (round-half-up via exact
small-significand integer adds), clamped below at 1/2, plus a two-threshold
step for the {0, 1/64} region. Verified bit-exact vs the jax reference modulo
~1-ulp boundary fuzz (~1 flipped element per 16M weights on the actual data).
"""
import sys
import numpy as np

for _p in ("/opt/trn_rl_repo", "/root/.axon_site/_ro/trn_rl_repo"):
    if _p not in sys.path:
        sys.path.append(_p)

N_CORES = 8
B, IN, H, OUT = 8192, 1024, 4096, 1024
BS = B // N_CORES          # batch shard per core
HS = H // N_CORES          # hidden-row shard per core (w1/w2/w3)
OS = OUT // N_CORES        # out-row shard per core (w4)

# FP4 codebook-derived threshold constants (g-space = 3*norm), f64 precision
_FP4_POS = np.array([0.0, 0.0052083333, 0.6666667, 1.0, 0.3333333, 0.5,
                     0.1666667, 0.25], dtype=np.float32)
_CS = np.sort(_FP4_POS).astype(np.float64)
_TL = float(np.float32(3.0 * (_CS[0] + _CS[1]) / 2.0))
_TH = float(np.float32(3.0 * (_CS[1] + _CS[2]) / 2.0))
LO_BITS = int(np.float32(1.0 / 64).view(np.uint32))   # 0x3C800000
BIG_BITS = 0x40400000                                  # bits of 3.0


def _i32(x):
    return int(np.uint32(x).view(np.int32))


_CACHED = {}

# weight dims per layer: (rows of W == dout, k == contraction)
WDIMS = {1: (H, IN), 2: (H, H), 3: (H, H), 4: (OUT, H)}
FDQ = 512          # dequant chunk free-size (fp32 elems per partition)
NBQ = FDQ // 64    # fp4 blocks per chunk


def _build_nc(taps=False):
    import concourse.bass as bass
    import concourse.mybir as mybir
    import concourse.tile as tile
    from concourse import bacc

    dt = mybir.dt
    Alu = mybir.AluOpType
    Act = mybir.ActivationFunctionType

    nc = bacc.Bacc("TRN2", target_bir_lowering=False, debug=False,
                   num_devices=N_CORES)

    # ---- I/O ----
    xs = nc.dram_tensor("xs", [BS, IN], dt.float32, kind="ExternalInput")
    w_in = {
        1: nc.dram_tensor("w1s", [HS, IN], dt.float32, kind="ExternalInput"),
        2: nc.dram_tensor("w2s", [HS, H], dt.float32, kind="ExternalInput"),
        3: nc.dram_tensor("w3s", [HS, H], dt.float32, kind="ExternalInput"),
        4: nc.dram_tensor("w4s", [OS, H], dt.float32, kind="ExternalInput"),
    }
    b_in = {
        1: nc.dram_tensor("b1", [H // 128, 128], dt.float32, kind="ExternalInput"),
        2: nc.dram_tensor("b2", [H // 128, 128], dt.float32, kind="ExternalInput"),
        3: nc.dram_tensor("b3", [H // 128, 128], dt.float32, kind="ExternalInput"),
        4: nc.dram_tensor("b4", [OUT // 128, 128], dt.float32, kind="ExternalInput"),
    }
    y_out = nc.dram_tensor("y", [OUT, BS], dt.float32, kind="ExternalOutput")

    # ---- internal DRAM: dequantized W.T-layout shards + AllGather outputs ----
    dq_shard = {}
    dq_full = {}
    for l, (dout, k) in WDIMS.items():
        rs = dout // N_CORES
        dq_shard[l] = nc.dram_tensor(f"dqs{l}", [rs // 128, k, 128], dt.float16)
        dq_full[l] = nc.dram_tensor(f"dqf{l}", [dout // 128, k, 128], dt.float16,
                                    addr_space="Shared")

    tap_t = {}
    if taps:
        for l, (dout, k) in WDIMS.items():
            tap_t[f"dq{l}"] = nc.dram_tensor(f"tap_dq{l}", [dout // 128, k, 128],
                                             dt.float16, kind="ExternalOutput")
        tap_t["a0"] = nc.dram_tensor("tap_a0", [128, (IN // 128) * BS],
                                     dt.float16, kind="ExternalOutput")
        for l in (1, 2, 3):
            tap_t[f"a{l}"] = nc.dram_tensor(f"tap_a{l}", [128, (H // 128) * BS],
                                            dt.float16, kind="ExternalOutput")

    with tile.TileContext(nc) as tc:
        with (
            tc.tile_pool(name="const", bufs=1) as cpool,
            tc.tile_pool(name="bias", bufs=1) as bpool,
            tc.tile_pool(name="xload", bufs=2) as xpool,
            tc.tile_pool(name="a0", bufs=1) as a0pool,
            tc.tile_pool(name="acts", bufs=2) as apool,
            tc.tile_pool(name="dqin", bufs=2) as dqin_pool,
            tc.tile_pool(name="dqtmp", bufs=1) as dqtmp_pool,
            tc.tile_pool(name="dqout", bufs=2) as dqout_pool,
            tc.tile_pool(name="wt", bufs=6) as wpool,
            tc.tile_pool(name="psum", bufs=4, space="PSUM") as pspool,
        ):
            # int32 constants for scalar_tensor_tensor scalars
            c_lo = cpool.tile([128, 1], dt.int32)
            nc.vector.memset(c_lo[:], _i32(LO_BITS))
            c_half = cpool.tile([128, 1], dt.int32)
            nc.vector.memset(c_half[:], _i32(0x3F000000))
            c_sign = cpool.tile([128, 1], dt.int32)
            nc.vector.memset(c_sign[:], _i32(0x80000000))

            # ---- biases -> SBUF [128, ntiles] (SP stream, startup) ----
            b_sb = {}
            for l, (dout, _k) in WDIMS.items():
                nj = dout // 128
                bt = bpool.tile([128, nj], dt.float32, tag=f"bias{l}")
                for j in range(nj):
                    nc.sync.dma_start(bt[:, j:j + 1], b_in[l][j])
                b_sb[l] = bt

            # ---- x load + cast fp16 + transpose -> A0 (ACT stream) ----
            a_cur = a0pool.tile([128, IN // 128, BS], dt.float16)
            for bt_i in range(BS // 128):
                xt = xpool.tile([128, IN], dt.float32, tag="xt")
                nc.scalar.dma_start(xt[:], xs[bt_i * 128:(bt_i + 1) * 128, :])
                xh = xpool.tile([128, IN], dt.float16, tag="xh")
                nc.vector.tensor_copy(xh[:], xt[:])
                for jk in range(IN // 128):
                    nc.scalar.dma_start_transpose(
                        a_cur[:, jk, bt_i * 128:(bt_i + 1) * 128],
                        xh[:, jk * 128:(jk + 1) * 128])

            def emit_dq_tile(l, r, cix):
                """One dequant chunk: [128 h-rows, FDQ k] of weight l's shard."""
                w = dqin_pool.tile([128, NBQ, 64], dt.float32, tag="dqw")
                nc.scalar.dma_start(
                    w[:],
                    w_in[l][r * 128:(r + 1) * 128, cix * FDQ:(cix + 1) * FDQ]
                    .rearrange("p (b i) -> p b i", i=64))
                scale = dqtmp_pool.tile([128, NBQ, 1], dt.float32, tag="scale")
                nc.vector.tensor_reduce(scale[:], w[:], axis=mybir.AxisListType.X,
                                        op=Alu.max, apply_absolute_value=True)
                recip = dqtmp_pool.tile([128, NBQ, 1], dt.float32, tag="recip")
                nc.vector.reciprocal(recip[:], scale[:])
                s3 = dqtmp_pool.tile([128, NBQ, 1], dt.float32, tag="s3")
                nc.vector.tensor_scalar_mul(s3[:], scale[:], 1.0 / 3.0)
                g = dqtmp_pool.tile([128, NBQ, 64], dt.float32, tag="g")
                nc.vector.scalar_tensor_tensor(
                    g[:], w[:], 3.0, recip[:].broadcast_to((128, NBQ, 64)),
                    op0=Alu.mult, op1=Alu.mult)
                gi = g[:].bitcast(dt.int32)
                # NOTE: DVE ops must never write in-place onto their own input
                # (dual-port perf modes race), and int adds must keep few
                # significant bits (the int ALU path is fp32-internal).
                ta = dqtmp_pool.tile([128, NBQ, 64], dt.int32, tag="ta")
                nc.vector.tensor_scalar(ta[:], gi, _i32(0x7FFFFFFF), None,
                                        op0=Alu.bitwise_and)  # m0 = |g| bits
                tb = dqtmp_pool.tile([128, NBQ, 64], dt.int32, tag="tb")
                nc.vector.tensor_scalar(tb[:], ta[:], _i32(0xFFC00000), None,
                                        op0=Alu.bitwise_and)  # trunc
                tc_ = dqtmp_pool.tile([128, NBQ, 64], dt.int32, tag="tc")
                nc.vector.tensor_scalar(tc_[:], ta[:], _i32(0x00200000), _i32(1),
                                        op0=Alu.bitwise_and,
                                        op1=Alu.logical_shift_left)  # half-bit<<1
                te = dqtmp_pool.tile([128, NBQ, 64], dt.int32, tag="te")
                nc.vector.tensor_tensor(te[:], tb[:], tc_[:],
                                        op=Alu.add)  # r2a (exact: 10+1 sig bits)
                af = ta[:].bitcast(dt.float32)  # |g| as float
                nc.vector.tensor_scalar(tb[:], af, _TL, 1.0,
                                        op0=Alu.is_le, op1=Alu.subtract)  # M1
                td = dqtmp_pool.tile([128, NBQ, 64], dt.int32, tag="td")
                nc.vector.tensor_scalar(td[:], af, _TH, 1.0,
                                        op0=Alu.is_le, op1=Alu.subtract)  # M2
                nc.vector.tensor_scalar(ta[:], td[:], _i32(BIG_BITS), None,
                                        op0=Alu.bitwise_and)  # S2 (m0 dead)
                nc.vector.scalar_tensor_tensor(
                    tc_[:], tb[:], c_lo[:], ta[:],
                    op0=Alu.bitwise_and, op1=Alu.bitwise_or)  # sel
                nc.vector.scalar_tensor_tensor(
                    tb[:], te[:], c_half[:], tc_[:],
                    op0=Alu.max, op1=Alu.min)  # mag
                nc.vector.scalar_tensor_tensor(
                    ta[:], gi, c_sign[:], tb[:],
                    op0=Alu.bitwise_and, op1=Alu.bitwise_or)  # signed
                dq = dqout_pool.tile([128, NBQ, 64], dt.float16, tag="dq")
                nc.vector.tensor_tensor(
                    dq[:], ta[:].bitcast(dt.float32),
                    s3[:].broadcast_to((128, NBQ, 64)), op=Alu.mult)
                # transpose-store into W.T layout shard
                dqt = dqout_pool.tile([128, FDQ // 128, 128], dt.float16,
                                      tag="dqt")
                dqf = dq[:].rearrange("p b i -> p (b i)")
                for ck in range(FDQ // 128):
                    nc.scalar.dma_start_transpose(
                        dqt[:, ck, :], dqf[:, ck * 128:(ck + 1) * 128])
                nc.scalar.dma_start(
                    dq_shard[l][r, cix * FDQ:(cix + 1) * FDQ, :]
                    .rearrange("(c p) h -> p c h", p=128),
                    dqt[:])

            def dq_tiles_of(l):
                rs = WDIMS[l][0] // N_CORES
                for r in range(rs // 128):
                    for cix in range(WDIMS[l][1] // FDQ):
                        yield (l, r, cix)

            def emit_allgather(l):
                nc.gpsimd.collective_compute(
                    "AllGather", Alu.bypass,
                    replica_groups=[list(range(N_CORES))],
                    ins=[dq_shard[l][:]],
                    outs=[dq_full[l][:]],
                )
                if taps:
                    nc.scalar.dma_start(tap_t[f"dq{l}"][:], dq_full[l][:])

            # dequant w1 up front, AllGather it
            for t in dq_tiles_of(1):
                emit_dq_tile(*t)
            emit_allgather(1)

            if taps:
                nc.scalar.dma_start(tap_t["a0"][:],
                                    a_cur[:].rearrange("p j b -> p (j b)"))

            # ---- matmul layers; layer l interleaves dequant of weight l+1 ----
            for l, (dout, K) in WDIMS.items():
                nj = dout // 128
                nk = K // 128
                out_dt = dt.float32 if l == 4 else dt.float16
                a_next = apool.tile([128, nj, BS], out_dt, tag="acts")
                # distribute next weight's dq tiles across this layer's j-loop
                pending = list(dq_tiles_of(l + 1)) if l < 4 else []
                half = nk // 2
                for j in range(nj):
                    wts = []
                    for i0 in (0, half):
                        wt_h = wpool.tile([128, half, 128], dt.float16, tag="wt")
                        nc.sync.dma_start(
                            wt_h[:],
                            dq_full[l][j, i0 * 128:(i0 + half) * 128, :]
                            .rearrange("(i p) h -> p i h", p=128))
                        wts.append(wt_h)
                    ps = []
                    for _n in range(BS // 512):
                        ps_t = pspool.tile([128, 512], dt.float32, tag="ps")
                        ps.append(ps_t)
                    for i in range(nk):
                        for n in range(BS // 512):
                            nc.tensor.matmul(
                                ps[n][:], wts[i // half][:, i % half, :],
                                a_cur[:, i, n * 512:(n + 1) * 512],
                                start=(i == 0), stop=(i == nk - 1))
                    for n in range(BS // 512):
                        if l == 4:
                            nc.scalar.activation(
                                a_next[:, j, n * 512:(n + 1) * 512], ps[n][:],
                                Act.Sigmoid, bias=b_sb[l][:, j:j + 1], scale=1.0)
                        else:
                            # relu(z + b) = (z add b) max 0, fused on DVE
                            nc.vector.tensor_scalar(
                                a_next[:, j, n * 512:(n + 1) * 512], ps[n][:],
                                b_sb[l][:, j:j + 1], 0.0,
                                op0=Alu.add, op1=Alu.max)
                    # interleave next weight's dequant chunks
                    n_emit = ((j + 1) * len(pending) + nj - 1) // nj - \
                             (j * len(pending) + nj - 1) // nj if pending else 0
                    done = (j * len(pending) + nj - 1) // nj if pending else 0
                    for t in pending[done:done + n_emit]:
                        emit_dq_tile(*t)
                if l < 4:
                    emit_allgather(l + 1)
                if taps and l < 4:
                    nc.scalar.dma_start(tap_t[f"a{l}"][:],
                                        a_next[:].rearrange("p j b -> p (j b)"))
                a_cur = a_next

            # ---- output: feature-major [OUT, BS] (SP stream) ----
            for j in range(OUT // 128):
                nc.sync.dma_start(y_out[j * 128:(j + 1) * 128, :], a_cur[:, j, :])

    nc.compile()
    return nc


def _get_nc():
    if "nc" not in _CACHED:
        _CACHED["nc"] = _build_nc()
    return _CACHED["nc"]


def kernel(**inputs):
    from concourse.bass_utils import run_bass_kernel_spmd

    x = np.ascontiguousarray(np.asarray(inputs["x"], dtype=np.float32))
    ws = {l: np.ascontiguousarray(np.asarray(inputs[f"w{l}"], dtype=np.float32))
          for l in (1, 2, 3, 4)}
    bs = {l: np.ascontiguousarray(
        np.asarray(inputs[f"b{l}"], dtype=np.float32).reshape(-1, 128))
        for l in (1, 2, 3, 4)}

    nc = _get_nc()
    in_maps = []
    for c in range(N_CORES):
        m = {
            "xs": x[c * BS:(c + 1) * BS],
            "w1s": ws[1][c * HS:(c + 1) * HS],
            "w2s": ws[2][c * HS:(c + 1) * HS],
            "w3s": ws[3][c * HS:(c + 1) * HS],
            "w4s": ws[4][c * OS:(c + 1) * OS],
            "b1": bs[1], "b2": bs[2], "b3": bs[3], "b4": bs[4],
        }
        in_maps.append(m)

    res = run_bass_kernel_spmd(nc, in_maps, list(range(N_CORES)))
    out = np.empty((B, OUT), dtype=np.float32)
    for c in range(N_CORES):
        out[c * BS:(c + 1) * BS] = res.results[c]["y"].T
    return out


if __name__ == "__main__":
    rng = np.random.default_rng(0)
    ins = {
        "x": rng.standard_normal((B, IN)).astype(np.float32),
        "w1": (rng.standard_normal((H, IN)) * 0.1).astype(np.float32),
        "b1": np.zeros(H, np.float32),
        "w2": (rng.standard_normal((H, H)) * 0.1).astype(np.float32),
        "b2": np.zeros(H, np.float32),
        "w3": (rng.standard_normal((H, H)) * 0.1).astype(np.float32),
        "b3": np.zeros(OUT if False else H, np.float32),
        "w4": (rng.standard_normal((OUT, H)) * 0.1).astype(np.float32),
        "b4": np.zeros(OUT, np.float32),
    }
    y = kernel(**ins)
    print("kernel ran, output shape", y.shape, "mean", float(y.mean()))


# revision 13
# speedup vs baseline: 1.5490x; 1.1712x over previous
"""FP4Net (bnb-FP4 quantize-dequantize 4-layer MLP) Trainium2 kernel.

Strategy (8 NeuronCores):
  - Data-parallel over batch for the matmuls: each core handles 1024 of 8192 rows.
  - FP4 quant-dequant of the weights is sharded 8x across cores (by output-row
    blocks, keeping the 64-elem FP4 blocks intact), computed exactly with fp32
    bit tricks on the vector engine, stored transposed (W.T layout) in fp16,
    then AllGathered so every core has all dequantized weights.
  - 4 chained fp16 matmul layers (fp32 PSUM accumulate); bias+ReLU epilogues on
    the vector engine, sigmoid on the scalar engine; activations resident in
    SBUF feature-major.
  - Engine streams are kept separate to avoid sequencer head-of-line blocking:
    SP: bias loads + weight-strip loads + output stores;
    ACT: x staging + all dequant-phase DMAs + final sigmoid;
    DVE: dequant compute + ReLU epilogues (emission-interleaved);
    GpSimd: AllGathers. Dequant of weight l+1 overlaps layer l's matmuls.

Rounding trick: with g = 3*w/scale, the bnb FP4 codebook {0, 1/192, 1/6, 1/4,
1/3, 1/2, 2/3, 1} maps to {0, 1/64, 1/2, 3/4, 1, 3/2, 2, 3}: round-to-nearest
over that set == round g to 1 stored mantissa bit (round-half-up via exact
small-significand integer adds), clamped below at 1/2, plus a two-threshold
step for the {0, 1/64} region. Verified bit-exact vs the jax reference modulo
~1-ulp boundary fuzz (~1 flipped element per 16M weights on the actual data).
"""
import sys
import numpy as np

for _p in ("/opt/trn_rl_repo", "/root/.axon_site/_ro/trn_rl_repo"):
    if _p not in sys.path:
        sys.path.append(_p)

N_CORES = 8
B, IN, H, OUT = 8192, 1024, 4096, 1024
BS = B // N_CORES          # batch shard per core
HS = H // N_CORES          # hidden-row shard per core (w1/w2/w3)
OS = OUT // N_CORES        # out-row shard per core (w4)

# FP4 codebook-derived threshold constants (g-space = 3*norm), f64 precision
_FP4_POS = np.array([0.0, 0.0052083333, 0.6666667, 1.0, 0.3333333, 0.5,
                     0.1666667, 0.25], dtype=np.float32)
_CS = np.sort(_FP4_POS).astype(np.float64)
_TL = float(np.float32(3.0 * (_CS[0] + _CS[1]) / 2.0))
_TH = float(np.float32(3.0 * (_CS[1] + _CS[2]) / 2.0))
LO_BITS = int(np.float32(1.0 / 64).view(np.uint32))   # 0x3C800000
BIG_BITS = 0x40400000                                  # bits of 3.0


def _i32(x):
    return int(np.uint32(x).view(np.int32))


_CACHED = {}

# weight dims per layer: (rows of W == dout, k == contraction)
WDIMS = {1: (H, IN), 2: (H, H), 3: (H, H), 4: (OUT, H)}
FDQ = 512          # dequant chunk free-size (fp32 elems per partition)
NBQ = FDQ // 64    # fp4 blocks per chunk


def _build_nc(taps=False):
    import concourse.bass as bass
    import concourse.mybir as mybir
    import concourse.tile as tile
    from concourse import bacc

    dt = mybir.dt
    Alu = mybir.AluOpType
    Act = mybir.ActivationFunctionType

    nc = bacc.Bacc("TRN2", target_bir_lowering=False, debug=False,
                   num_devices=N_CORES)

    # ---- I/O ----
    xs = nc.dram_tensor("xst", [IN, BS], dt.float32, kind="ExternalInput")
    w_in = {
        1: nc.dram_tensor("w1s", [HS, IN], dt.float32, kind="ExternalInput"),
        2: nc.dram_tensor("w2s", [HS, H], dt.float32, kind="ExternalInput"),
        3: nc.dram_tensor("w3s", [HS, H], dt.float32, kind="ExternalInput"),
        4: nc.dram_tensor("w4s", [OS, H], dt.float32, kind="ExternalInput"),
    }
    b_in = {
        1: nc.dram_tensor("b1", [128, H // 128], dt.float32, kind="ExternalInput"),
        2: nc.dram_tensor("b2", [128, H // 128], dt.float32, kind="ExternalInput"),
        3: nc.dram_tensor("b3", [128, H // 128], dt.float32, kind="ExternalInput"),
        4: nc.dram_tensor("b4", [128, OUT // 128], dt.float32, kind="ExternalInput"),
    }
    y_out = nc.dram_tensor("y", [OUT, BS], dt.float32, kind="ExternalOutput")

    # ---- internal DRAM: dequantized W.T-layout shards + AllGather outputs ----
    dq_shard = {}
    dq_full = {}
    for l, (dout, k) in WDIMS.items():
        rs = dout // N_CORES
        dq_shard[l] = nc.dram_tensor(f"dqs{l}", [rs // 128, k, 128], dt.float16)
        dq_full[l] = nc.dram_tensor(f"dqf{l}", [dout // 128, k, 128], dt.float16,
                                    addr_space="Shared")

    tap_t = {}
    if taps:
        for l, (dout, k) in WDIMS.items():
            tap_t[f"dq{l}"] = nc.dram_tensor(f"tap_dq{l}", [dout // 128, k, 128],
                                             dt.float16, kind="ExternalOutput")
        tap_t["a0"] = nc.dram_tensor("tap_a0", [128, (IN // 128) * BS],
                                     dt.float16, kind="ExternalOutput")
        for l in (1, 2, 3):
            tap_t[f"a{l}"] = nc.dram_tensor(f"tap_a{l}", [128, (H // 128) * BS],
                                            dt.float16, kind="ExternalOutput")

    with tile.TileContext(nc) as tc:
        with (
            tc.tile_pool(name="const", bufs=1) as cpool,
            tc.tile_pool(name="bias", bufs=1) as bpool,
            tc.tile_pool(name="xload", bufs=2) as xpool,
            tc.tile_pool(name="a0", bufs=1) as a0pool,
            tc.tile_pool(name="acts", bufs=2) as apool,
            tc.tile_pool(name="dqin", bufs=2) as dqin_pool,
            tc.tile_pool(name="dqtmp", bufs=1) as dqtmp_pool,
            tc.tile_pool(name="dqout", bufs=2) as dqout_pool,
            tc.tile_pool(name="wt", bufs=6) as wpool,
            tc.tile_pool(name="psum", bufs=4, space="PSUM") as pspool,
        ):
            # int32 constants for scalar_tensor_tensor scalars
            c_lo = cpool.tile([128, 1], dt.int32)
            nc.vector.memset(c_lo[:], _i32(LO_BITS))
            c_half = cpool.tile([128, 1], dt.int32)
            nc.vector.memset(c_half[:], _i32(0x3F000000))
            c_sign = cpool.tile([128, 1], dt.int32)
            nc.vector.memset(c_sign[:], _i32(0x80000000))

            # ---- biases -> SBUF [128, ntiles] (SP stream, startup) ----
            b_sb = {}
            for l, (dout, _k) in WDIMS.items():
                nj = dout // 128
                bt = bpool.tile([128, nj], dt.float32, tag=f"bias{l}")
                nc.sync.dma_start(bt[:], b_in[l][:])
                b_sb[l] = bt

            # ---- x (host-pre-transposed) load + cast fp16 -> A0 ----
            a_cur = a0pool.tile([128, IN // 128, BS], dt.float16)
            for jk in range(IN // 128):
                xt = xpool.tile([128, BS], dt.float32, tag="xt")
                nc.scalar.dma_start(xt[:], xs[jk * 128:(jk + 1) * 128, :])
                nc.vector.tensor_copy(a_cur[:, jk, :], xt[:])

            def emit_dq_tile(l, r, cix):
                """One dequant chunk: [128 h-rows, FDQ k] of weight l's shard."""
                w = dqin_pool.tile([128, NBQ, 64], dt.float32, tag="dqw")
                nc.scalar.dma_start(
                    w[:],
                    w_in[l][r * 128:(r + 1) * 128, cix * FDQ:(cix + 1) * FDQ]
                    .rearrange("p (b i) -> p b i", i=64))
                scale = dqtmp_pool.tile([128, NBQ, 1], dt.float32, tag="scale")
                nc.vector.tensor_reduce(scale[:], w[:], axis=mybir.AxisListType.X,
                                        op=Alu.max, apply_absolute_value=True)
                recip = dqtmp_pool.tile([128, NBQ, 1], dt.float32, tag="recip")
                nc.vector.reciprocal(recip[:], scale[:])
                s3 = dqtmp_pool.tile([128, NBQ, 1], dt.float32, tag="s3")
                nc.vector.tensor_scalar_mul(s3[:], scale[:], 1.0 / 3.0)
                g = dqtmp_pool.tile([128, NBQ, 64], dt.float32, tag="g")
                nc.vector.scalar_tensor_tensor(
                    g[:], w[:], 3.0, recip[:].broadcast_to((128, NBQ, 64)),
                    op0=Alu.mult, op1=Alu.mult)
                gi = g[:].bitcast(dt.int32)
                # NOTE: DVE ops must never write in-place onto their own input
                # (dual-port perf modes race), and int adds must keep few
                # significant bits (the int ALU path is fp32-internal).
                ta = dqtmp_pool.tile([128, NBQ, 64], dt.int32, tag="ta")
                nc.vector.tensor_scalar(ta[:], gi, _i32(0x7FFFFFFF), None,
                                        op0=Alu.bitwise_and)  # m0 = |g| bits
                tb = dqtmp_pool.tile([128, NBQ, 64], dt.int32, tag="tb")
                nc.vector.tensor_scalar(tb[:], ta[:], _i32(0xFFC00000), None,
                                        op0=Alu.bitwise_and)  # trunc
                tc_ = dqtmp_pool.tile([128, NBQ, 64], dt.int32, tag="tc")
                nc.vector.tensor_scalar(tc_[:], ta[:], _i32(0x00200000), _i32(1),
                                        op0=Alu.bitwise_and,
                                        op1=Alu.logical_shift_left)  # half-bit<<1
                te = dqtmp_pool.tile([128, NBQ, 64], dt.int32, tag="te")
                nc.vector.tensor_tensor(te[:], tb[:], tc_[:],
                                        op=Alu.add)  # r2a (exact: 10+1 sig bits)
                af = ta[:].bitcast(dt.float32)  # |g| as float
                nc.vector.tensor_scalar(tb[:], af, _TL, 1.0,
                                        op0=Alu.is_le, op1=Alu.subtract)  # M1
                td = dqtmp_pool.tile([128, NBQ, 64], dt.int32, tag="td")
                nc.vector.tensor_scalar(td[:], af, _TH, 1.0,
                                        op0=Alu.is_le, op1=Alu.subtract)  # M2
                nc.vector.tensor_scalar(ta[:], td[:], _i32(BIG_BITS), None,
                                        op0=Alu.bitwise_and)  # S2 (m0 dead)
                nc.vector.scalar_tensor_tensor(
                    tc_[:], tb[:], c_lo[:], ta[:],
                    op0=Alu.bitwise_and, op1=Alu.bitwise_or)  # sel
                nc.vector.scalar_tensor_tensor(
                    tb[:], te[:], c_half[:], tc_[:],
                    op0=Alu.max, op1=Alu.min)  # mag
                nc.vector.scalar_tensor_tensor(
                    ta[:], gi, c_sign[:], tb[:],
                    op0=Alu.bitwise_and, op1=Alu.bitwise_or)  # signed
                dq = dqout_pool.tile([128, NBQ, 64], dt.float16, tag="dq")
                nc.vector.tensor_tensor(
                    dq[:], ta[:].bitcast(dt.float32),
                    s3[:].broadcast_to((128, NBQ, 64)), op=Alu.mult)
                # transpose-store into W.T layout shard
                dqt = dqout_pool.tile([128, FDQ // 128, 128], dt.float16,
                                      tag="dqt")
                nc.scalar.dma_start_transpose(
                    dqt[:], dq[:].rearrange("p b i -> p (b i)"))
                nc.scalar.dma_start(
                    dq_shard[l][r, cix * FDQ:(cix + 1) * FDQ, :]
                    .rearrange("(c p) h -> p c h", p=128),
                    dqt[:])

            def dq_tiles_of(l):
                rs = WDIMS[l][0] // N_CORES
                for r in range(rs // 128):
                    for cix in range(WDIMS[l][1] // FDQ):
                        yield (l, r, cix)

            def emit_allgather(l):
                nc.gpsimd.collective_compute(
                    "AllGather", Alu.bypass,
                    replica_groups=[list(range(N_CORES))],
                    ins=[dq_shard[l][:]],
                    outs=[dq_full[l][:]],
                )
                if taps:
                    nc.scalar.dma_start(tap_t[f"dq{l}"][:], dq_full[l][:])

            # dequant w1 up front, AllGather it
            for t in dq_tiles_of(1):
                emit_dq_tile(*t)
            emit_allgather(1)

            if taps:
                nc.scalar.dma_start(tap_t["a0"][:],
                                    a_cur[:].rearrange("p j b -> p (j b)"))

            # ---- matmul layers; layer l interleaves dequant of weight l+1 ----
            for l, (dout, K) in WDIMS.items():
                nj = dout // 128
                nk = K // 128
                out_dt = dt.float32 if l == 4 else dt.float16
                a_next = apool.tile([128, nj, BS], out_dt, tag="acts")
                # distribute next weight's dq tiles across this layer's j-loop
                pending = list(dq_tiles_of(l + 1)) if l < 4 else []
                half = nk // 2
                for j in range(nj):
                    wts = []
                    for i0 in (0, half):
                        wt_h = wpool.tile([128, half, 128], dt.float16, tag="wt")
                        nc.sync.dma_start(
                            wt_h[:],
                            dq_full[l][j, i0 * 128:(i0 + half) * 128, :]
                            .rearrange("(i p) h -> p i h", p=128))
                        wts.append(wt_h)
                    ps = []
                    for _n in range(BS // 512):
                        ps_t = pspool.tile([128, 512], dt.float32, tag="ps")
                        ps.append(ps_t)
                    for i in range(nk):
                        for n in range(BS // 512):
                            nc.tensor.matmul(
                                ps[n][:], wts[i // half][:, i % half, :],
                                a_cur[:, i, n * 512:(n + 1) * 512],
                                start=(i == 0), stop=(i == nk - 1))
                    for n in range(BS // 512):
                        if l == 4:
                            nc.scalar.activation(
                                a_next[:, j, n * 512:(n + 1) * 512], ps[n][:],
                                Act.Sigmoid, bias=b_sb[l][:, j:j + 1], scale=1.0)
                        else:
                            # relu(z + b) = (z add b) max 0, fused on DVE
                            nc.vector.tensor_scalar(
                                a_next[:, j, n * 512:(n + 1) * 512], ps[n][:],
                                b_sb[l][:, j:j + 1], 0.0,
                                op0=Alu.add, op1=Alu.max)
                    # interleave next weight's dequant chunks
                    n_emit = ((j + 1) * len(pending) + nj - 1) // nj - \
                             (j * len(pending) + nj - 1) // nj if pending else 0
                    done = (j * len(pending) + nj - 1) // nj if pending else 0
                    for t in pending[done:done + n_emit]:
                        emit_dq_tile(*t)
                if l < 4:
                    emit_allgather(l + 1)
                if taps and l < 4:
                    nc.scalar.dma_start(tap_t[f"a{l}"][:],
                                        a_next[:].rearrange("p j b -> p (j b)"))
                a_cur = a_next

            # ---- output: feature-major [OUT, BS] (SP stream) ----
            for j in range(OUT // 128):
                nc.sync.dma_start(y_out[j * 128:(j + 1) * 128, :], a_cur[:, j, :])

    nc.compile()
    return nc


def _get_nc():
    if "nc" not in _CACHED:
        _CACHED["nc"] = _build_nc()
    return _CACHED["nc"]


def kernel(**inputs):
    from concourse.bass_utils import run_bass_kernel_spmd

    x = np.asarray(inputs["x"], dtype=np.float32)
    ws = {l: np.ascontiguousarray(np.asarray(inputs[f"w{l}"], dtype=np.float32))
          for l in (1, 2, 3, 4)}
    bs = {l: np.ascontiguousarray(
        np.asarray(inputs[f"b{l}"], dtype=np.float32).reshape(-1, 128).T)
        for l in (1, 2, 3, 4)}

    nc = _get_nc()
    in_maps = []
    for c in range(N_CORES):
        m = {
            "xst": np.ascontiguousarray(x[c * BS:(c + 1) * BS].T),
            "w1s": ws[1][c * HS:(c + 1) * HS],
            "w2s": ws[2][c * HS:(c + 1) * HS],
            "w3s": ws[3][c * HS:(c + 1) * HS],
            "w4s": ws[4][c * OS:(c + 1) * OS],
            "b1": bs[1], "b2": bs[2], "b3": bs[3], "b4": bs[4],
        }
        in_maps.append(m)

    res = run_bass_kernel_spmd(nc, in_maps, list(range(N_CORES)))
    out = np.empty((B, OUT), dtype=np.float32)
    for c in range(N_CORES):
        out[c * BS:(c + 1) * BS] = res.results[c]["y"].T
    return out


if __name__ == "__main__":
    rng = np.random.default_rng(0)
    ins = {
        "x": rng.standard_normal((B, IN)).astype(np.float32),
        "w1": (rng.standard_normal((H, IN)) * 0.1).astype(np.float32),
        "b1": np.zeros(H, np.float32),
        "w2": (rng.standard_normal((H, H)) * 0.1).astype(np.float32),
        "b2": np.zeros(H, np.float32),
        "w3": (rng.standard_normal((H, H)) * 0.1).astype(np.float32),
        "b3": np.zeros(OUT if False else H, np.float32),
        "w4": (rng.standard_normal((OUT, H)) * 0.1).astype(np.float32),
        "b4": np.zeros(OUT, np.float32),
    }
    y = kernel(**ins)
    print("kernel ran, output shape", y.shape, "mean", float(y.mean()))


# revision 14
# speedup vs baseline: 1.6725x; 1.0797x over previous
"""FP4Net (bnb-FP4 quantize-dequantize 4-layer MLP) Trainium2 kernel.

Strategy (8 NeuronCores):
  - Data-parallel over batch for the matmuls: each core handles 1024 of 8192 rows.
  - FP4 quant-dequant of the weights is sharded 8x across cores (by output-row
    blocks, keeping the 64-elem FP4 blocks intact), computed exactly with fp32
    bit tricks on the vector engine, stored transposed (W.T layout) in fp16,
    then AllGathered so every core has all dequantized weights.
  - 4 chained fp16 matmul layers (fp32 PSUM accumulate); bias+ReLU epilogues on
    the vector engine, sigmoid on the scalar engine; activations resident in
    SBUF feature-major.
  - Engine streams are kept separate to avoid sequencer head-of-line blocking:
    SP: bias loads + weight-strip loads + output stores;
    ACT: x staging + all dequant-phase DMAs + final sigmoid;
    DVE: dequant compute + ReLU epilogues (emission-interleaved);
    GpSimd: AllGathers. Dequant of weight l+1 overlaps layer l's matmuls.

Rounding trick: with g = 3*w/scale, the bnb FP4 codebook {0, 1/192, 1/6, 1/4,
1/3, 1/2, 2/3, 1} maps to {0, 1/64, 1/2, 3/4, 1, 3/2, 2, 3}: round-to-nearest
over that set == round g to 1 stored mantissa bit (round-half-up via exact
small-significand integer adds), clamped below at 1/2, plus a two-threshold
step for the {0, 1/64} region. Verified bit-exact vs the jax reference modulo
~1-ulp boundary fuzz (~1 flipped element per 16M weights on the actual data).
"""
import sys
import numpy as np

for _p in ("/opt/trn_rl_repo", "/root/.axon_site/_ro/trn_rl_repo"):
    if _p not in sys.path:
        sys.path.append(_p)

N_CORES = 8
B, IN, H, OUT = 8192, 1024, 4096, 1024
BS = B // N_CORES          # batch shard per core
HS = H // N_CORES          # hidden-row shard per core (w1/w2/w3)
OS = OUT // N_CORES        # out-row shard per core (w4)

# FP4 codebook-derived threshold constants (g-space = 3*norm), f64 precision
_FP4_POS = np.array([0.0, 0.0052083333, 0.6666667, 1.0, 0.3333333, 0.5,
                     0.1666667, 0.25], dtype=np.float32)
_CS = np.sort(_FP4_POS).astype(np.float64)
_TL = float(np.float32(3.0 * (_CS[0] + _CS[1]) / 2.0))
_TH = float(np.float32(3.0 * (_CS[1] + _CS[2]) / 2.0))
LO_BITS = int(np.float32(1.0 / 64).view(np.uint32))   # 0x3C800000
BIG_BITS = 0x40400000                                  # bits of 3.0


def _i32(x):
    return int(np.uint32(x).view(np.int32))


_CACHED = {}

# weight dims per layer: (rows of W == dout, k == contraction)
WDIMS = {1: (H, IN), 2: (H, H), 3: (H, H), 4: (OUT, H)}
FDQ = 512          # dequant chunk free-size (fp32 elems per partition)
NBQ = FDQ // 64    # fp4 blocks per chunk


def _build_nc(taps=False):
    import concourse.bass as bass
    import concourse.mybir as mybir
    import concourse.tile as tile
    from concourse import bacc

    dt = mybir.dt
    Alu = mybir.AluOpType
    Act = mybir.ActivationFunctionType

    nc = bacc.Bacc("TRN2", target_bir_lowering=False, debug=False,
                   num_devices=N_CORES)

    # ---- I/O ----
    xs = nc.dram_tensor("xst", [IN, BS], dt.float32, kind="ExternalInput")
    w_in = {
        1: nc.dram_tensor("w1s", [HS, IN], dt.float32, kind="ExternalInput"),
        2: nc.dram_tensor("w2s", [HS, H], dt.float32, kind="ExternalInput"),
        3: nc.dram_tensor("w3s", [HS, H], dt.float32, kind="ExternalInput"),
        4: nc.dram_tensor("w4s", [OS, H], dt.float32, kind="ExternalInput"),
    }
    b_in = {
        1: nc.dram_tensor("b1", [128, H // 128], dt.float32, kind="ExternalInput"),
        2: nc.dram_tensor("b2", [128, H // 128], dt.float32, kind="ExternalInput"),
        3: nc.dram_tensor("b3", [128, H // 128], dt.float32, kind="ExternalInput"),
        4: nc.dram_tensor("b4", [128, OUT // 128], dt.float32, kind="ExternalInput"),
    }
    y_out = nc.dram_tensor("y", [OUT, BS], dt.float32, kind="ExternalOutput")

    # ---- internal DRAM: dequantized W.T-layout shards + AllGather outputs ----
    dq_shard = {}
    dq_full = {}   # l -> list of gathered half tensors (1 for w4)
    for l, (dout, k) in WDIMS.items():
        rs = dout // N_CORES
        nrt = rs // 128
        dq_shard[l] = nc.dram_tensor(f"dqs{l}", [nrt, k, 128], dt.float16)
        if nrt > 1:
            dq_full[l] = [
                nc.dram_tensor(f"dqf{l}{h}", [N_CORES * nrt // 2, k, 128],
                               dt.float16, addr_space="Shared")
                for h in range(2)]
        else:
            dq_full[l] = [nc.dram_tensor(f"dqf{l}0", [N_CORES * nrt, k, 128],
                                         dt.float16, addr_space="Shared")]

    tap_t = {}
    if taps:
        tap_t["a0"] = nc.dram_tensor("tap_a0", [128, (IN // 128) * BS],
                                     dt.float16, kind="ExternalOutput")
        for l in (1, 2, 3):
            tap_t[f"a{l}"] = nc.dram_tensor(f"tap_a{l}", [128, (H // 128) * BS],
                                            dt.float16, kind="ExternalOutput")

    with tile.TileContext(nc) as tc:
        with (
            tc.tile_pool(name="const", bufs=1) as cpool,
            tc.tile_pool(name="bias", bufs=1) as bpool,
            tc.tile_pool(name="xload", bufs=2) as xpool,
            tc.tile_pool(name="a0", bufs=1) as a0pool,
            tc.tile_pool(name="acts", bufs=2) as apool,
            tc.tile_pool(name="dqin", bufs=2) as dqin_pool,
            tc.tile_pool(name="dqtmp", bufs=1) as dqtmp_pool,
            tc.tile_pool(name="dqout", bufs=2) as dqout_pool,
            tc.tile_pool(name="wt", bufs=6) as wpool,
            tc.tile_pool(name="psum", bufs=8, space="PSUM") as pspool,
        ):
            # int32 constants for scalar_tensor_tensor scalars
            c_lo = cpool.tile([128, 1], dt.int32)
            nc.vector.memset(c_lo[:], _i32(LO_BITS))
            c_half = cpool.tile([128, 1], dt.int32)
            nc.vector.memset(c_half[:], _i32(0x3F000000))
            c_sign = cpool.tile([128, 1], dt.int32)
            nc.vector.memset(c_sign[:], _i32(0x80000000))

            # ---- biases -> SBUF [128, ntiles] (SP stream, startup) ----
            b_sb = {}
            for l, (dout, _k) in WDIMS.items():
                nj = dout // 128
                bt = bpool.tile([128, nj], dt.float32, tag=f"bias{l}")
                nc.sync.dma_start(bt[:], b_in[l][:])
                b_sb[l] = bt

            def emit_dq_tile(l, r, cix):
                """One dequant chunk: [128 h-rows, FDQ k] of weight l's shard."""
                w = dqin_pool.tile([128, NBQ, 64], dt.float32, tag="dqw")
                nc.scalar.dma_start(
                    w[:],
                    w_in[l][r * 128:(r + 1) * 128, cix * FDQ:(cix + 1) * FDQ]
                    .rearrange("p (b i) -> p b i", i=64))
                scale = dqtmp_pool.tile([128, NBQ, 1], dt.float32, tag="scale")
                nc.vector.tensor_reduce(scale[:], w[:], axis=mybir.AxisListType.X,
                                        op=Alu.max, apply_absolute_value=True)
                recip = dqtmp_pool.tile([128, NBQ, 1], dt.float32, tag="recip")
                nc.vector.reciprocal(recip[:], scale[:])
                s3 = dqtmp_pool.tile([128, NBQ, 1], dt.float32, tag="s3")
                nc.vector.tensor_scalar_mul(s3[:], scale[:], 1.0 / 3.0)
                g = dqtmp_pool.tile([128, NBQ, 64], dt.float32, tag="g")
                nc.vector.scalar_tensor_tensor(
                    g[:], w[:], 3.0, recip[:].broadcast_to((128, NBQ, 64)),
                    op0=Alu.mult, op1=Alu.mult)
                gi = g[:].bitcast(dt.int32)
                # NOTE: DVE ops must never write in-place onto their own input
                # (dual-port perf modes race), and int adds must keep few
                # significant bits (the int ALU path is fp32-internal).
                ta = dqtmp_pool.tile([128, NBQ, 64], dt.int32, tag="ta")
                nc.vector.tensor_scalar(ta[:], gi, _i32(0x7FFFFFFF), None,
                                        op0=Alu.bitwise_and)  # m0 = |g| bits
                tb = dqtmp_pool.tile([128, NBQ, 64], dt.int32, tag="tb")
                nc.vector.tensor_scalar(tb[:], ta[:], _i32(0xFFC00000), None,
                                        op0=Alu.bitwise_and)  # trunc
                tc_ = dqtmp_pool.tile([128, NBQ, 64], dt.int32, tag="tc")
                nc.vector.tensor_scalar(tc_[:], ta[:], _i32(0x00200000), _i32(1),
                                        op0=Alu.bitwise_and,
                                        op1=Alu.logical_shift_left)  # half-bit<<1
                te = dqtmp_pool.tile([128, NBQ, 64], dt.int32, tag="te")
                nc.vector.tensor_tensor(te[:], tb[:], tc_[:],
                                        op=Alu.add)  # r2a (exact: 10+1 sig bits)
                af = ta[:].bitcast(dt.float32)  # |g| as float
                nc.vector.tensor_scalar(tb[:], af, _TL, 1.0,
                                        op0=Alu.is_le, op1=Alu.subtract)  # M1
                td = dqtmp_pool.tile([128, NBQ, 64], dt.int32, tag="td")
                nc.vector.tensor_scalar(td[:], af, _TH, 1.0,
                                        op0=Alu.is_le, op1=Alu.subtract)  # M2
                nc.vector.tensor_scalar(ta[:], td[:], _i32(BIG_BITS), None,
                                        op0=Alu.bitwise_and)  # S2 (m0 dead)
                nc.vector.scalar_tensor_tensor(
                    tc_[:], tb[:], c_lo[:], ta[:],
                    op0=Alu.bitwise_and, op1=Alu.bitwise_or)  # sel
                nc.vector.scalar_tensor_tensor(
                    tb[:], te[:], c_half[:], tc_[:],
                    op0=Alu.max, op1=Alu.min)  # mag
                nc.vector.scalar_tensor_tensor(
                    ta[:], gi, c_sign[:], tb[:],
                    op0=Alu.bitwise_and, op1=Alu.bitwise_or)  # signed
                dq = dqout_pool.tile([128, NBQ, 64], dt.float16, tag="dq")
                nc.vector.tensor_tensor(
                    dq[:], ta[:].bitcast(dt.float32),
                    s3[:].broadcast_to((128, NBQ, 64)), op=Alu.mult)
                # transpose-store into W.T layout shard
                dqt = dqout_pool.tile([128, FDQ // 128, 128], dt.float16,
                                      tag="dqt")
                nc.scalar.dma_start_transpose(
                    dqt[:], dq[:].rearrange("p b i -> p (b i)"))
                nc.scalar.dma_start(
                    dq_shard[l][r, cix * FDQ:(cix + 1) * FDQ, :]
                    .rearrange("(c p) h -> p c h", p=128),
                    dqt[:])

            def dq_tiles_of(l):
                rs = WDIMS[l][0] // N_CORES
                for r in range(rs // 128):
                    for cix in range(WDIMS[l][1] // FDQ):
                        yield (l, r, cix)

            def emit_allgather_half(l, h):
                nrt = WDIMS[l][0] // N_CORES // 128
                if nrt > 1:
                    ins = dq_shard[l][h * (nrt // 2):(h + 1) * (nrt // 2)]
                else:
                    ins = dq_shard[l][:]
                nc.gpsimd.collective_compute(
                    "AllGather", Alu.bypass,
                    replica_groups=[list(range(N_CORES))],
                    ins=[ins],
                    outs=[dq_full[l][h][:]],
                )

            def dq_emitter(l):
                """Generator: send n -> emits next n dq tiles of weight l,
                issuing each half's AllGather as soon as its tiles are done."""
                tiles = list(dq_tiles_of(l))
                nhalf = len(dq_full[l])
                per_half = len(tiles) // nhalf
                done = 0
                while done < len(tiles):
                    n = yield
                    for _ in range(n or 1):
                        if done >= len(tiles):
                            break
                        emit_dq_tile(*tiles[done])
                        done += 1
                        if done % per_half == 0:
                            emit_allgather_half(l, done // per_half - 1)
                while True:
                    yield

            # dequant w1 up front (both halves + AGs)
            em1 = dq_emitter(1)
            next(em1)
            em1.send(len(list(dq_tiles_of(1))))

            # ---- x (host-pre-transposed) load + cast fp16 -> A0 ----
            a_cur = a0pool.tile([128, IN // 128, BS], dt.float16)
            for jk in range(IN // 128):
                xt = xpool.tile([128, BS], dt.float32, tag="xt")
                nc.scalar.dma_start(xt[:], xs[jk * 128:(jk + 1) * 128, :])
                nc.vector.tensor_copy(a_cur[:, jk, :], xt[:])

            if taps:
                nc.scalar.dma_start(tap_t["a0"][:],
                                    a_cur[:].rearrange("p j b -> p (j b)"))

            # ---- matmul layers; layer l interleaves dequant of weight l+1 ----
            for l, (dout, K) in WDIMS.items():
                nj = dout // 128
                nk = K // 128
                nhalf = len(dq_full[l])
                out_dt = dt.float32 if l == 4 else dt.float16
                a_next = apool.tile([128, nj, BS], out_dt, tag="acts")
                emitter = None
                if l < 4:
                    emitter = dq_emitter(l + 1)
                    next(emitter)
                half = nk // 2
                # consume j in AG-half order: all first-half tiles, then second
                if nhalf == 2:
                    j_order = [c * 4 + hh * 2 + r
                               for hh in range(2) for c in range(N_CORES)
                               for r in range(2)]
                else:
                    j_order = list(range(nj))
                for j in j_order:
                    if nhalf == 2:
                        hsel, lt = (0, (j // 4) * 2 + j % 4) if j % 4 < 2 \
                            else (1, (j // 4) * 2 + j % 4 - 2)
                    else:
                        hsel, lt = 0, j
                    src_t = dq_full[l][hsel]
                    wts = []
                    for i0 in (0, half):
                        wt_h = wpool.tile([128, half, 128], dt.float16, tag="wt")
                        nc.sync.dma_start(
                            wt_h[:],
                            src_t[lt, i0 * 128:(i0 + half) * 128, :]
                            .rearrange("(i p) h -> p i h", p=128))
                        wts.append(wt_h)
                    ps = []
                    for _n in range(BS // 512):
                        ps_t = pspool.tile([128, 512], dt.float32, tag="ps")
                        ps.append(ps_t)
                    for i in range(nk):
                        for n in range(BS // 512):
                            nc.tensor.matmul(
                                ps[n][:], wts[i // half][:, i % half, :],
                                a_cur[:, i, n * 512:(n + 1) * 512],
                                start=(i == 0), stop=(i == nk - 1))
                    for n in range(BS // 512):
                        if l == 4:
                            nc.scalar.activation(
                                a_next[:, j, n * 512:(n + 1) * 512], ps[n][:],
                                Act.Sigmoid, bias=b_sb[l][:, j:j + 1], scale=1.0)
                        else:
                            # relu(z + b) = (z add b) max 0, fused on DVE
                            nc.vector.tensor_scalar(
                                a_next[:, j, n * 512:(n + 1) * 512], ps[n][:],
                                b_sb[l][:, j:j + 1], 0.0,
                                op0=Alu.add, op1=Alu.max)
                    # interleave next weight's dequant (front-loaded, 2 per j)
                    if emitter is not None:
                        emitter.send(2)
                if taps and l < 4:
                    nc.scalar.dma_start(tap_t[f"a{l}"][:],
                                        a_next[:].rearrange("p j b -> p (j b)"))
                a_cur = a_next

            # ---- output: feature-major [OUT, BS] (SP stream) ----
            for j in range(OUT // 128):
                nc.sync.dma_start(y_out[j * 128:(j + 1) * 128, :], a_cur[:, j, :])

    nc.compile()
    return nc


def _get_nc():
    if "nc" not in _CACHED:
        _CACHED["nc"] = _build_nc()
    return _CACHED["nc"]


def kernel(**inputs):
    from concourse.bass_utils import run_bass_kernel_spmd

    x = np.asarray(inputs["x"], dtype=np.float32)
    ws = {l: np.ascontiguousarray(np.asarray(inputs[f"w{l}"], dtype=np.float32))
          for l in (1, 2, 3, 4)}
    bs = {l: np.ascontiguousarray(
        np.asarray(inputs[f"b{l}"], dtype=np.float32).reshape(-1, 128).T)
        for l in (1, 2, 3, 4)}

    nc = _get_nc()
    in_maps = []
    for c in range(N_CORES):
        m = {
            "xst": np.ascontiguousarray(x[c * BS:(c + 1) * BS].T),
            "w1s": ws[1][c * HS:(c + 1) * HS],
            "w2s": ws[2][c * HS:(c + 1) * HS],
            "w3s": ws[3][c * HS:(c + 1) * HS],
            "w4s": ws[4][c * OS:(c + 1) * OS],
            "b1": bs[1], "b2": bs[2], "b3": bs[3], "b4": bs[4],
        }
        in_maps.append(m)

    res = run_bass_kernel_spmd(nc, in_maps, list(range(N_CORES)))
    out = np.empty((B, OUT), dtype=np.float32)
    for c in range(N_CORES):
        out[c * BS:(c + 1) * BS] = res.results[c]["y"].T
    return out


if __name__ == "__main__":
    rng = np.random.default_rng(0)
    ins = {
        "x": rng.standard_normal((B, IN)).astype(np.float32),
        "w1": (rng.standard_normal((H, IN)) * 0.1).astype(np.float32),
        "b1": np.zeros(H, np.float32),
        "w2": (rng.standard_normal((H, H)) * 0.1).astype(np.float32),
        "b2": np.zeros(H, np.float32),
        "w3": (rng.standard_normal((H, H)) * 0.1).astype(np.float32),
        "b3": np.zeros(OUT if False else H, np.float32),
        "w4": (rng.standard_normal((OUT, H)) * 0.1).astype(np.float32),
        "b4": np.zeros(OUT, np.float32),
    }
    y = kernel(**ins)
    print("kernel ran, output shape", y.shape, "mean", float(y.mean()))


# revision 15
# speedup vs baseline: 1.7249x; 1.0313x over previous
"""FP4Net (bnb-FP4 quantize-dequantize 4-layer MLP) Trainium2 kernel.

Strategy (8 NeuronCores):
  - Data-parallel over batch for the matmuls: each core handles 1024 of 8192 rows.
  - FP4 quant-dequant of the weights is sharded 8x across cores (by output-row
    blocks, keeping the 64-elem FP4 blocks intact), computed exactly with fp32
    bit tricks on the vector engine, stored transposed (W.T layout) in fp16,
    then AllGathered so every core has all dequantized weights.
  - 4 chained fp16 matmul layers (fp32 PSUM accumulate); bias+ReLU epilogues on
    the vector engine, sigmoid on the scalar engine; activations resident in
    SBUF feature-major.
  - Engine streams are kept separate to avoid sequencer head-of-line blocking:
    SP: bias loads + weight-strip loads + output stores;
    ACT: x staging + all dequant-phase DMAs + final sigmoid;
    DVE: dequant compute + ReLU epilogues (emission-interleaved);
    GpSimd: AllGathers. Dequant of weight l+1 overlaps layer l's matmuls.

Rounding trick: with g = 3*w/scale, the bnb FP4 codebook {0, 1/192, 1/6, 1/4,
1/3, 1/2, 2/3, 1} maps to {0, 1/64, 1/2, 3/4, 1, 3/2, 2, 3}: round-to-nearest
over that set == round g to 1 stored mantissa bit (round-half-up via exact
small-significand integer adds), clamped below at 1/2, plus a two-threshold
step for the {0, 1/64} region. Verified bit-exact vs the jax reference modulo
~1-ulp boundary fuzz (~1 flipped element per 16M weights on the actual data).
"""
import sys
import numpy as np

for _p in ("/opt/trn_rl_repo", "/root/.axon_site/_ro/trn_rl_repo"):
    if _p not in sys.path:
        sys.path.append(_p)

N_CORES = 8
B, IN, H, OUT = 8192, 1024, 4096, 1024
BS = B // N_CORES          # batch shard per core
HS = H // N_CORES          # hidden-row shard per core (w1/w2/w3)
OS = OUT // N_CORES        # out-row shard per core (w4)

# FP4 codebook-derived threshold constants (g-space = 3*norm), f64 precision
_FP4_POS = np.array([0.0, 0.0052083333, 0.6666667, 1.0, 0.3333333, 0.5,
                     0.1666667, 0.25], dtype=np.float32)
_CS = np.sort(_FP4_POS).astype(np.float64)
_TL = float(np.float32(3.0 * (_CS[0] + _CS[1]) / 2.0))
_TH = float(np.float32(3.0 * (_CS[1] + _CS[2]) / 2.0))
LO_BITS = int(np.float32(1.0 / 64).view(np.uint32))   # 0x3C800000
BIG_BITS = 0x40400000                                  # bits of 3.0


def _i32(x):
    return int(np.uint32(x).view(np.int32))


_CACHED = {}

# weight dims per layer: (rows of W == dout, k == contraction)
WDIMS = {1: (H, IN), 2: (H, H), 3: (H, H), 4: (OUT, H)}
FDQ = 512          # dequant chunk free-size (fp32 elems per partition)
NBQ = FDQ // 64    # fp4 blocks per chunk


def _build_nc(taps=False):
    import concourse.bass as bass
    import concourse.mybir as mybir
    import concourse.tile as tile
    from concourse import bacc

    dt = mybir.dt
    Alu = mybir.AluOpType
    Act = mybir.ActivationFunctionType

    nc = bacc.Bacc("TRN2", target_bir_lowering=False, debug=False,
                   num_devices=N_CORES)

    # ---- I/O ----
    xs = nc.dram_tensor("xst", [IN, BS], dt.float32, kind="ExternalInput")
    w_in = {
        1: nc.dram_tensor("w1s", [HS, IN], dt.float32, kind="ExternalInput"),
        2: nc.dram_tensor("w2s", [HS, H], dt.float32, kind="ExternalInput"),
        3: nc.dram_tensor("w3s", [HS, H], dt.float32, kind="ExternalInput"),
        4: nc.dram_tensor("w4s", [OS, H], dt.float32, kind="ExternalInput"),
    }
    b_in = {
        1: nc.dram_tensor("b1", [128, H // 128], dt.float32, kind="ExternalInput"),
        2: nc.dram_tensor("b2", [128, H // 128], dt.float32, kind="ExternalInput"),
        3: nc.dram_tensor("b3", [128, H // 128], dt.float32, kind="ExternalInput"),
        4: nc.dram_tensor("b4", [128, OUT // 128], dt.float32, kind="ExternalInput"),
    }
    y_out = nc.dram_tensor("y", [OUT, BS], dt.float32, kind="ExternalOutput")

    # ---- internal DRAM: dequantized W.T-layout shards + AllGather outputs ----
    dq_shard = {}
    dq_full = {}   # l -> list of gathered half tensors (1 for w4)
    for l, (dout, k) in WDIMS.items():
        rs = dout // N_CORES
        nrt = rs // 128
        dq_shard[l] = nc.dram_tensor(f"dqs{l}", [nrt, k, 128], dt.float16)
        if nrt > 1:
            dq_full[l] = [
                nc.dram_tensor(f"dqf{l}{h}", [N_CORES * nrt // 2, k, 128],
                               dt.float16, addr_space="Shared")
                for h in range(2)]
        else:
            dq_full[l] = [nc.dram_tensor(f"dqf{l}0", [N_CORES * nrt, k, 128],
                                         dt.float16, addr_space="Shared")]

    tap_t = {}
    if taps:
        tap_t["a0"] = nc.dram_tensor("tap_a0", [128, (IN // 128) * BS],
                                     dt.float16, kind="ExternalOutput")
        for l in (1, 2, 3):
            tap_t[f"a{l}"] = nc.dram_tensor(f"tap_a{l}", [128, (H // 128) * BS],
                                            dt.float16, kind="ExternalOutput")

    with tile.TileContext(nc) as tc:
        with (
            tc.tile_pool(name="const", bufs=1) as cpool,
            tc.tile_pool(name="bias", bufs=1) as bpool,
            tc.tile_pool(name="xload", bufs=2) as xpool,
            tc.tile_pool(name="a0", bufs=1) as a0pool,
            tc.tile_pool(name="acts", bufs=2) as apool,
            tc.tile_pool(name="dqin", bufs=2) as dqin_pool,
            tc.tile_pool(name="dqtmp", bufs=1) as dqtmp_pool,
            tc.tile_pool(name="dqout", bufs=2) as dqout_pool,
            tc.tile_pool(name="wt", bufs=6) as wpool,
            tc.tile_pool(name="psum", bufs=8, space="PSUM") as pspool,
        ):
            # int32 constants for scalar_tensor_tensor scalars
            c_lo = cpool.tile([128, 1], dt.int32)
            nc.vector.memset(c_lo[:], _i32(LO_BITS))
            c_half = cpool.tile([128, 1], dt.int32)
            nc.vector.memset(c_half[:], _i32(0x3F000000))
            c_sign = cpool.tile([128, 1], dt.int32)
            nc.vector.memset(c_sign[:], _i32(0x80000000))

            # ---- biases -> SBUF [128, ntiles] (SP stream, startup) ----
            b_sb = {}
            for l, (dout, _k) in WDIMS.items():
                nj = dout // 128
                bt = bpool.tile([128, nj], dt.float32, tag=f"bias{l}")
                nc.sync.dma_start(bt[:], b_in[l][:])
                b_sb[l] = bt

            def emit_dq_tile(l, r, cix):
                """One dequant chunk: [128 h-rows, FDQ k] of weight l's shard."""
                w = dqin_pool.tile([128, NBQ, 64], dt.float32, tag="dqw")
                nc.gpsimd.dma_start(
                    w[:],
                    w_in[l][r * 128:(r + 1) * 128, cix * FDQ:(cix + 1) * FDQ]
                    .rearrange("p (b i) -> p b i", i=64))
                scale = dqtmp_pool.tile([128, NBQ, 1], dt.float32, tag="scale")
                nc.vector.tensor_reduce(scale[:], w[:], axis=mybir.AxisListType.X,
                                        op=Alu.max, apply_absolute_value=True)
                recip = dqtmp_pool.tile([128, NBQ, 1], dt.float32, tag="recip")
                nc.vector.reciprocal(recip[:], scale[:])
                s3 = dqtmp_pool.tile([128, NBQ, 1], dt.float32, tag="s3")
                nc.vector.tensor_scalar_mul(s3[:], scale[:], 1.0 / 3.0)
                g = dqtmp_pool.tile([128, NBQ, 64], dt.float32, tag="g")
                nc.vector.scalar_tensor_tensor(
                    g[:], w[:], 3.0, recip[:].broadcast_to((128, NBQ, 64)),
                    op0=Alu.mult, op1=Alu.mult)
                gi = g[:].bitcast(dt.int32)
                # NOTE: DVE ops must never write in-place onto their own input
                # (dual-port perf modes race), and int adds must keep few
                # significant bits (the int ALU path is fp32-internal).
                ta = dqtmp_pool.tile([128, NBQ, 64], dt.int32, tag="ta")
                nc.vector.tensor_scalar(ta[:], gi, _i32(0x7FFFFFFF), None,
                                        op0=Alu.bitwise_and)  # m0 = |g| bits
                tb = dqtmp_pool.tile([128, NBQ, 64], dt.int32, tag="tb")
                nc.vector.tensor_scalar(tb[:], ta[:], _i32(0xFFC00000), None,
                                        op0=Alu.bitwise_and)  # trunc
                tc_ = dqtmp_pool.tile([128, NBQ, 64], dt.int32, tag="tc")
                nc.vector.tensor_scalar(tc_[:], ta[:], _i32(0x00200000), _i32(1),
                                        op0=Alu.bitwise_and,
                                        op1=Alu.logical_shift_left)  # half-bit<<1
                te = dqtmp_pool.tile([128, NBQ, 64], dt.int32, tag="te")
                nc.vector.tensor_tensor(te[:], tb[:], tc_[:],
                                        op=Alu.add)  # r2a (exact: 10+1 sig bits)
                af = ta[:].bitcast(dt.float32)  # |g| as float
                # M1L = (|g|>TL)*LO_BITS, M2B = (|g|>TH)*BIG_BITS -- the float
                # products are exact (consts have <=5 significant bits)
                nc.vector.tensor_scalar(tb[:], af, _TL, float(LO_BITS),
                                        op0=Alu.is_gt, op1=Alu.mult)
                td = dqtmp_pool.tile([128, NBQ, 64], dt.int32, tag="td")
                nc.vector.tensor_scalar(td[:], af, _TH, float(BIG_BITS),
                                        op0=Alu.is_gt, op1=Alu.mult)
                nc.vector.tensor_tensor(tc_[:], tb[:], td[:],
                                        op=Alu.add)  # sel (disjoint bits)
                nc.vector.scalar_tensor_tensor(
                    tb[:], te[:], c_half[:], tc_[:],
                    op0=Alu.max, op1=Alu.min)  # mag
                nc.vector.scalar_tensor_tensor(
                    ta[:], gi, c_sign[:], tb[:],
                    op0=Alu.bitwise_and, op1=Alu.bitwise_or)  # signed
                dq = dqout_pool.tile([128, NBQ, 64], dt.float16, tag="dq")
                nc.vector.tensor_tensor(
                    dq[:], ta[:].bitcast(dt.float32),
                    s3[:].broadcast_to((128, NBQ, 64)), op=Alu.mult)
                # transpose-store into W.T layout shard
                dqt = dqout_pool.tile([128, FDQ // 128, 128], dt.float16,
                                      tag="dqt")
                nc.sync.dma_start_transpose(
                    dqt[:], dq[:].rearrange("p b i -> p (b i)"))
                nc.gpsimd.dma_start(
                    dq_shard[l][r, cix * FDQ:(cix + 1) * FDQ, :]
                    .rearrange("(c p) h -> p c h", p=128),
                    dqt[:])

            def dq_tiles_of(l):
                rs = WDIMS[l][0] // N_CORES
                for r in range(rs // 128):
                    for cix in range(WDIMS[l][1] // FDQ):
                        yield (l, r, cix)

            def emit_allgather_half(l, h):
                nrt = WDIMS[l][0] // N_CORES // 128
                if nrt > 1:
                    ins = dq_shard[l][h * (nrt // 2):(h + 1) * (nrt // 2)]
                else:
                    ins = dq_shard[l][:]
                nc.gpsimd.collective_compute(
                    "AllGather", Alu.bypass,
                    replica_groups=[list(range(N_CORES))],
                    ins=[ins],
                    outs=[dq_full[l][h][:]],
                )

            def dq_emitter(l):
                """Generator: send n -> emits next n dq tiles of weight l,
                issuing each half's AllGather as soon as its tiles are done."""
                tiles = list(dq_tiles_of(l))
                nhalf = len(dq_full[l])
                per_half = len(tiles) // nhalf
                done = 0
                while done < len(tiles):
                    n = yield
                    for _ in range(n or 1):
                        if done >= len(tiles):
                            break
                        emit_dq_tile(*tiles[done])
                        done += 1
                        if done % per_half == 0:
                            emit_allgather_half(l, done // per_half - 1)
                while True:
                    yield

            # dequant w1 up front (both halves + AGs)
            em1 = dq_emitter(1)
            next(em1)
            em1.send(len(list(dq_tiles_of(1))))

            # ---- x (host-pre-transposed) load + cast fp16 -> A0 ----
            a_cur = a0pool.tile([128, IN // 128, BS], dt.float16)
            for jk in range(IN // 128):
                xt = xpool.tile([128, BS], dt.float32, tag="xt")
                nc.gpsimd.dma_start(xt[:], xs[jk * 128:(jk + 1) * 128, :])
                nc.vector.tensor_copy(a_cur[:, jk, :], xt[:])

            if taps:
                nc.scalar.dma_start(tap_t["a0"][:],
                                    a_cur[:].rearrange("p j b -> p (j b)"))

            # ---- matmul layers; layer l interleaves dequant of weight l+1 ----
            for l, (dout, K) in WDIMS.items():
                nj = dout // 128
                nk = K // 128
                nhalf = len(dq_full[l])
                out_dt = dt.float32 if l == 4 else dt.float16
                a_next = apool.tile([128, nj, BS], out_dt, tag="acts")
                emitter = None
                if l < 4:
                    emitter = dq_emitter(l + 1)
                    next(emitter)
                half = nk // 2
                # consume j in AG-half order: all first-half tiles, then second
                if nhalf == 2:
                    j_order = [c * 4 + hh * 2 + r
                               for hh in range(2) for c in range(N_CORES)
                               for r in range(2)]
                else:
                    j_order = list(range(nj))
                for j in j_order:
                    if nhalf == 2:
                        hsel, lt = (0, (j // 4) * 2 + j % 4) if j % 4 < 2 \
                            else (1, (j // 4) * 2 + j % 4 - 2)
                    else:
                        hsel, lt = 0, j
                    src_t = dq_full[l][hsel]
                    wts = []
                    for i0 in (0, half):
                        wt_h = wpool.tile([128, half, 128], dt.float16, tag="wt")
                        nc.sync.dma_start(
                            wt_h[:],
                            src_t[lt, i0 * 128:(i0 + half) * 128, :]
                            .rearrange("(i p) h -> p i h", p=128))
                        wts.append(wt_h)
                    ps = []
                    for _n in range(BS // 512):
                        ps_t = pspool.tile([128, 512], dt.float32, tag="ps")
                        ps.append(ps_t)
                    for i in range(nk):
                        for n in range(BS // 512):
                            nc.tensor.matmul(
                                ps[n][:], wts[i // half][:, i % half, :],
                                a_cur[:, i, n * 512:(n + 1) * 512],
                                start=(i == 0), stop=(i == nk - 1))
                    for n in range(BS // 512):
                        if l == 4:
                            nc.scalar.activation(
                                a_next[:, j, n * 512:(n + 1) * 512], ps[n][:],
                                Act.Sigmoid, bias=b_sb[l][:, j:j + 1], scale=1.0)
                        else:
                            nc.scalar.activation(
                                a_next[:, j, n * 512:(n + 1) * 512], ps[n][:],
                                Act.Relu, bias=b_sb[l][:, j:j + 1], scale=1.0)
                    # interleave next weight's dequant (front-loaded, 2 per j)
                    if emitter is not None:
                        emitter.send(2)
                if taps and l < 4:
                    nc.scalar.dma_start(tap_t[f"a{l}"][:],
                                        a_next[:].rearrange("p j b -> p (j b)"))
                a_cur = a_next

            # ---- output: feature-major [OUT, BS] (SP stream) ----
            for j in range(OUT // 128):
                nc.sync.dma_start(y_out[j * 128:(j + 1) * 128, :], a_cur[:, j, :])

    nc.compile()
    return nc


def _get_nc():
    if "nc" not in _CACHED:
        _CACHED["nc"] = _build_nc()
    return _CACHED["nc"]


def kernel(**inputs):
    from concourse.bass_utils import run_bass_kernel_spmd

    x = np.asarray(inputs["x"], dtype=np.float32)
    ws = {l: np.ascontiguousarray(np.asarray(inputs[f"w{l}"], dtype=np.float32))
          for l in (1, 2, 3, 4)}
    bs = {l: np.ascontiguousarray(
        np.asarray(inputs[f"b{l}"], dtype=np.float32).reshape(-1, 128).T)
        for l in (1, 2, 3, 4)}

    nc = _get_nc()
    in_maps = []
    for c in range(N_CORES):
        m = {
            "xst": np.ascontiguousarray(x[c * BS:(c + 1) * BS].T),
            "w1s": ws[1][c * HS:(c + 1) * HS],
            "w2s": ws[2][c * HS:(c + 1) * HS],
            "w3s": ws[3][c * HS:(c + 1) * HS],
            "w4s": ws[4][c * OS:(c + 1) * OS],
            "b1": bs[1], "b2": bs[2], "b3": bs[3], "b4": bs[4],
        }
        in_maps.append(m)

    res = run_bass_kernel_spmd(nc, in_maps, list(range(N_CORES)))
    out = np.empty((B, OUT), dtype=np.float32)
    for c in range(N_CORES):
        out[c * BS:(c + 1) * BS] = res.results[c]["y"].T
    return out


if __name__ == "__main__":
    rng = np.random.default_rng(0)
    ins = {
        "x": rng.standard_normal((B, IN)).astype(np.float32),
        "w1": (rng.standard_normal((H, IN)) * 0.1).astype(np.float32),
        "b1": np.zeros(H, np.float32),
        "w2": (rng.standard_normal((H, H)) * 0.1).astype(np.float32),
        "b2": np.zeros(H, np.float32),
        "w3": (rng.standard_normal((H, H)) * 0.1).astype(np.float32),
        "b3": np.zeros(OUT if False else H, np.float32),
        "w4": (rng.standard_normal((OUT, H)) * 0.1).astype(np.float32),
        "b4": np.zeros(OUT, np.float32),
    }
    y = kernel(**ins)
    print("kernel ran, output shape", y.shape, "mean", float(y.mean()))


# revision 16
# speedup vs baseline: 1.7280x; 1.0018x over previous
"""FP4Net (bnb-FP4 quantize-dequantize 4-layer MLP) Trainium2 kernel.

Strategy (8 NeuronCores):
  - Data-parallel over batch for the matmuls: each core handles 1024 of 8192 rows.
  - FP4 quant-dequant of the weights is sharded 8x across cores (by output-row
    blocks, keeping the 64-elem FP4 blocks intact), computed exactly with fp32
    bit tricks on the vector engine, stored transposed (W.T layout) in fp16,
    then AllGathered so every core has all dequantized weights.
  - 4 chained fp16 matmul layers (fp32 PSUM accumulate); bias+ReLU epilogues on
    the vector engine, sigmoid on the scalar engine; activations resident in
    SBUF feature-major.
  - Engine streams are kept separate to avoid sequencer head-of-line blocking:
    SP: bias loads + weight-strip loads + output stores;
    ACT: x staging + all dequant-phase DMAs + final sigmoid;
    DVE: dequant compute + ReLU epilogues (emission-interleaved);
    GpSimd: AllGathers. Dequant of weight l+1 overlaps layer l's matmuls.

Rounding trick: with g = 3*w/scale, the bnb FP4 codebook {0, 1/192, 1/6, 1/4,
1/3, 1/2, 2/3, 1} maps to {0, 1/64, 1/2, 3/4, 1, 3/2, 2, 3}: round-to-nearest
over that set == round g to 1 stored mantissa bit (round-half-up via exact
small-significand integer adds), clamped below at 1/2, plus a two-threshold
step for the {0, 1/64} region. Verified bit-exact vs the jax reference modulo
~1-ulp boundary fuzz (~1 flipped element per 16M weights on the actual data).
"""
import sys
import numpy as np

for _p in ("/opt/trn_rl_repo", "/root/.axon_site/_ro/trn_rl_repo"):
    if _p not in sys.path:
        sys.path.append(_p)

N_CORES = 8
B, IN, H, OUT = 8192, 1024, 4096, 1024
BS = B // N_CORES          # batch shard per core
HS = H // N_CORES          # hidden-row shard per core (w1/w2/w3)
OS = OUT // N_CORES        # out-row shard per core (w4)

# FP4 codebook-derived threshold constants (g-space = 3*norm), f64 precision
_FP4_POS = np.array([0.0, 0.0052083333, 0.6666667, 1.0, 0.3333333, 0.5,
                     0.1666667, 0.25], dtype=np.float32)
_CS = np.sort(_FP4_POS).astype(np.float64)
_TL = float(np.float32(3.0 * (_CS[0] + _CS[1]) / 2.0))
_TH = float(np.float32(3.0 * (_CS[1] + _CS[2]) / 2.0))
LO_BITS = int(np.float32(1.0 / 64).view(np.uint32))   # 0x3C800000
BIG_BITS = 0x40400000                                  # bits of 3.0


def _i32(x):
    return int(np.uint32(x).view(np.int32))


_CACHED = {}

# weight dims per layer: (rows of W == dout, k == contraction)
WDIMS = {1: (H, IN), 2: (H, H), 3: (H, H), 4: (OUT, H)}
FDQ = 512          # dequant chunk free-size (fp32 elems per partition)
NBQ = FDQ // 64    # fp4 blocks per chunk


def _build_nc(taps=False):
    import concourse.bass as bass
    import concourse.mybir as mybir
    import concourse.tile as tile
    from concourse import bacc

    dt = mybir.dt
    Alu = mybir.AluOpType
    Act = mybir.ActivationFunctionType

    nc = bacc.Bacc("TRN2", target_bir_lowering=False, debug=False,
                   num_devices=N_CORES)

    # ---- I/O ----
    xs = nc.dram_tensor("xst", [IN, BS], dt.float32, kind="ExternalInput")
    w_in = {
        1: nc.dram_tensor("w1s", [HS, IN], dt.float32, kind="ExternalInput"),
        2: nc.dram_tensor("w2s", [HS, H], dt.float32, kind="ExternalInput"),
        3: nc.dram_tensor("w3s", [HS, H], dt.float32, kind="ExternalInput"),
        4: nc.dram_tensor("w4s", [OS, H], dt.float32, kind="ExternalInput"),
    }
    b_in = {
        1: nc.dram_tensor("b1", [128, H // 128], dt.float32, kind="ExternalInput"),
        2: nc.dram_tensor("b2", [128, H // 128], dt.float32, kind="ExternalInput"),
        3: nc.dram_tensor("b3", [128, H // 128], dt.float32, kind="ExternalInput"),
        4: nc.dram_tensor("b4", [128, OUT // 128], dt.float32, kind="ExternalInput"),
    }
    y_out = nc.dram_tensor("y", [OUT, BS], dt.float32, kind="ExternalOutput")

    # ---- internal DRAM: dequantized W.T-layout shards + AllGather outputs ----
    dq_shard = {}
    dq_full = {}   # l -> list of gathered half tensors (1 for w4)
    for l, (dout, k) in WDIMS.items():
        rs = dout // N_CORES
        nrt = rs // 128
        dq_shard[l] = nc.dram_tensor(f"dqs{l}", [nrt, k, 128], dt.float16)
        if nrt > 1:
            dq_full[l] = [
                nc.dram_tensor(f"dqf{l}{h}", [N_CORES * nrt // 2, k, 128],
                               dt.float16, addr_space="Shared")
                for h in range(2)]
        else:
            dq_full[l] = [nc.dram_tensor(f"dqf{l}0", [N_CORES * nrt, k, 128],
                                         dt.float16, addr_space="Shared")]

    tap_t = {}
    if taps:
        tap_t["a0"] = nc.dram_tensor("tap_a0", [128, (IN // 128) * BS],
                                     dt.float16, kind="ExternalOutput")
        for l in (1, 2, 3):
            tap_t[f"a{l}"] = nc.dram_tensor(f"tap_a{l}", [128, (H // 128) * BS],
                                            dt.float16, kind="ExternalOutput")

    with tile.TileContext(nc) as tc:
        with (
            tc.tile_pool(name="const", bufs=1) as cpool,
            tc.tile_pool(name="bias", bufs=1) as bpool,
            tc.tile_pool(name="xload", bufs=2) as xpool,
            tc.tile_pool(name="a0", bufs=1) as a0pool,
            tc.tile_pool(name="acts", bufs=2) as apool,
            tc.tile_pool(name="dqin", bufs=4) as dqin_pool,
            tc.tile_pool(name="dqtmp", bufs=1) as dqtmp_pool,
            tc.tile_pool(name="dqout", bufs=3) as dqout_pool,
            tc.tile_pool(name="wt", bufs=6) as wpool,
            tc.tile_pool(name="psum", bufs=8, space="PSUM") as pspool,
        ):
            # int32 constants for scalar_tensor_tensor scalars
            c_lo = cpool.tile([128, 1], dt.int32)
            nc.vector.memset(c_lo[:], _i32(LO_BITS))
            c_half = cpool.tile([128, 1], dt.int32)
            nc.vector.memset(c_half[:], _i32(0x3F000000))
            c_sign = cpool.tile([128, 1], dt.int32)
            nc.vector.memset(c_sign[:], _i32(0x80000000))

            # ---- biases -> SBUF [128, ntiles] (SP stream, startup) ----
            b_sb = {}
            for l, (dout, _k) in WDIMS.items():
                nj = dout // 128
                bt = bpool.tile([128, nj], dt.float32, tag=f"bias{l}")
                nc.sync.dma_start(bt[:], b_in[l][:])
                b_sb[l] = bt

            def emit_dq_tile(l, r, cix):
                """One dequant chunk: [128 h-rows, FDQ k] of weight l's shard."""
                w = dqin_pool.tile([128, NBQ, 64], dt.float32, tag="dqw")
                nc.gpsimd.dma_start(
                    w[:],
                    w_in[l][r * 128:(r + 1) * 128, cix * FDQ:(cix + 1) * FDQ]
                    .rearrange("p (b i) -> p b i", i=64))
                scale = dqtmp_pool.tile([128, NBQ, 1], dt.float32, tag="scale")
                nc.vector.tensor_reduce(scale[:], w[:], axis=mybir.AxisListType.X,
                                        op=Alu.max, apply_absolute_value=True)
                recip = dqtmp_pool.tile([128, NBQ, 1], dt.float32, tag="recip")
                nc.vector.reciprocal(recip[:], scale[:])
                s3 = dqtmp_pool.tile([128, NBQ, 1], dt.float32, tag="s3")
                nc.vector.tensor_scalar_mul(s3[:], scale[:], 1.0 / 3.0)
                g = dqtmp_pool.tile([128, NBQ, 64], dt.float32, tag="g")
                nc.vector.scalar_tensor_tensor(
                    g[:], w[:], 3.0, recip[:].broadcast_to((128, NBQ, 64)),
                    op0=Alu.mult, op1=Alu.mult)
                gi = g[:].bitcast(dt.int32)
                # NOTE: DVE ops must never write in-place onto their own input
                # (dual-port perf modes race), and int adds must keep few
                # significant bits (the int ALU path is fp32-internal).
                ta = dqtmp_pool.tile([128, NBQ, 64], dt.int32, tag="ta")
                nc.vector.tensor_scalar(ta[:], gi, _i32(0x7FFFFFFF), None,
                                        op0=Alu.bitwise_and)  # m0 = |g| bits
                tb = dqtmp_pool.tile([128, NBQ, 64], dt.int32, tag="tb")
                nc.vector.tensor_scalar(tb[:], ta[:], _i32(0xFFC00000), None,
                                        op0=Alu.bitwise_and)  # trunc
                tc_ = dqtmp_pool.tile([128, NBQ, 64], dt.int32, tag="tc")
                nc.vector.tensor_scalar(tc_[:], ta[:], _i32(0x00200000), _i32(1),
                                        op0=Alu.bitwise_and,
                                        op1=Alu.logical_shift_left)  # half-bit<<1
                te = dqtmp_pool.tile([128, NBQ, 64], dt.int32, tag="te")
                nc.vector.tensor_tensor(te[:], tb[:], tc_[:],
                                        op=Alu.add)  # r2a (exact: 10+1 sig bits)
                af = ta[:].bitcast(dt.float32)  # |g| as float
                # M1L = (|g|>TL)*LO_BITS, M2B = (|g|>TH)*BIG_BITS -- the float
                # products are exact (consts have <=5 significant bits)
                nc.vector.tensor_scalar(tb[:], af, _TL, float(LO_BITS),
                                        op0=Alu.is_gt, op1=Alu.mult)
                td = dqtmp_pool.tile([128, NBQ, 64], dt.int32, tag="td")
                nc.vector.tensor_scalar(td[:], af, _TH, float(BIG_BITS),
                                        op0=Alu.is_gt, op1=Alu.mult)
                nc.vector.tensor_tensor(tc_[:], tb[:], td[:],
                                        op=Alu.add)  # sel (disjoint bits)
                nc.vector.scalar_tensor_tensor(
                    tb[:], te[:], c_half[:], tc_[:],
                    op0=Alu.max, op1=Alu.min)  # mag
                nc.vector.scalar_tensor_tensor(
                    ta[:], gi, c_sign[:], tb[:],
                    op0=Alu.bitwise_and, op1=Alu.bitwise_or)  # signed
                dq = dqout_pool.tile([128, NBQ, 64], dt.float16, tag="dq")
                nc.vector.tensor_tensor(
                    dq[:], ta[:].bitcast(dt.float32),
                    s3[:].broadcast_to((128, NBQ, 64)), op=Alu.mult)
                # transpose-store into W.T layout shard
                dqt = dqout_pool.tile([128, FDQ // 128, 128], dt.float16,
                                      tag="dqt")
                nc.sync.dma_start_transpose(
                    dqt[:], dq[:].rearrange("p b i -> p (b i)"))
                nc.gpsimd.dma_start(
                    dq_shard[l][r, cix * FDQ:(cix + 1) * FDQ, :]
                    .rearrange("(c p) h -> p c h", p=128),
                    dqt[:])

            def dq_tiles_of(l):
                rs = WDIMS[l][0] // N_CORES
                for r in range(rs // 128):
                    for cix in range(WDIMS[l][1] // FDQ):
                        yield (l, r, cix)

            def emit_allgather_half(l, h):
                nrt = WDIMS[l][0] // N_CORES // 128
                if nrt > 1:
                    ins = dq_shard[l][h * (nrt // 2):(h + 1) * (nrt // 2)]
                else:
                    ins = dq_shard[l][:]
                nc.gpsimd.collective_compute(
                    "AllGather", Alu.bypass,
                    replica_groups=[list(range(N_CORES))],
                    ins=[ins],
                    outs=[dq_full[l][h][:]],
                )

            def dq_emitter(l):
                """Generator: send n -> emits next n dq tiles of weight l,
                issuing each half's AllGather as soon as its tiles are done."""
                tiles = list(dq_tiles_of(l))
                nhalf = len(dq_full[l])
                per_half = len(tiles) // nhalf
                done = 0
                while done < len(tiles):
                    n = yield
                    for _ in range(n or 1):
                        if done >= len(tiles):
                            break
                        emit_dq_tile(*tiles[done])
                        done += 1
                        if done % per_half == 0:
                            emit_allgather_half(l, done // per_half - 1)
                while True:
                    yield

            # dequant w1 up front (both halves + AGs)
            em1 = dq_emitter(1)
            next(em1)
            em1.send(len(list(dq_tiles_of(1))))

            # ---- x (host-pre-transposed) load + cast fp16 -> A0 ----
            a_cur = a0pool.tile([128, IN // 128, BS], dt.float16)
            for jk in range(IN // 128):
                xt = xpool.tile([128, BS], dt.float32, tag="xt")
                nc.scalar.dma_start(xt[:], xs[jk * 128:(jk + 1) * 128, :])
                nc.vector.tensor_copy(a_cur[:, jk, :], xt[:])

            if taps:
                nc.scalar.dma_start(tap_t["a0"][:],
                                    a_cur[:].rearrange("p j b -> p (j b)"))

            # ---- matmul layers; layer l interleaves dequant of weight l+1 ----
            for l, (dout, K) in WDIMS.items():
                nj = dout // 128
                nk = K // 128
                nhalf = len(dq_full[l])
                out_dt = dt.float32 if l == 4 else dt.float16
                a_next = apool.tile([128, nj, BS], out_dt, tag="acts")
                emitter = None
                if l < 4:
                    emitter = dq_emitter(l + 1)
                    next(emitter)
                half = nk // 2
                # consume j in AG-half order: all first-half tiles, then second
                if nhalf == 2:
                    j_order = [c * 4 + hh * 2 + r
                               for hh in range(2) for c in range(N_CORES)
                               for r in range(2)]
                else:
                    j_order = list(range(nj))
                for j in j_order:
                    if nhalf == 2:
                        hsel, lt = (0, (j // 4) * 2 + j % 4) if j % 4 < 2 \
                            else (1, (j // 4) * 2 + j % 4 - 2)
                    else:
                        hsel, lt = 0, j
                    src_t = dq_full[l][hsel]
                    wts = []
                    for i0 in (0, half):
                        wt_h = wpool.tile([128, half, 128], dt.float16, tag="wt")
                        nc.sync.dma_start(
                            wt_h[:],
                            src_t[lt, i0 * 128:(i0 + half) * 128, :]
                            .rearrange("(i p) h -> p i h", p=128))
                        wts.append(wt_h)
                    ps = []
                    for _n in range(BS // 512):
                        ps_t = pspool.tile([128, 512], dt.float32, tag="ps")
                        ps.append(ps_t)
                    for i in range(nk):
                        for n in range(BS // 512):
                            nc.tensor.matmul(
                                ps[n][:], wts[i // half][:, i % half, :],
                                a_cur[:, i, n * 512:(n + 1) * 512],
                                start=(i == 0), stop=(i == nk - 1))
                    for n in range(BS // 512):
                        if l == 4:
                            nc.scalar.activation(
                                a_next[:, j, n * 512:(n + 1) * 512], ps[n][:],
                                Act.Sigmoid, bias=b_sb[l][:, j:j + 1], scale=1.0)
                        else:
                            nc.scalar.activation(
                                a_next[:, j, n * 512:(n + 1) * 512], ps[n][:],
                                Act.Relu, bias=b_sb[l][:, j:j + 1], scale=1.0)
                    # interleave next weight's dequant (front-loaded, 2 per j)
                    if emitter is not None:
                        emitter.send(2)
                if taps and l < 4:
                    nc.scalar.dma_start(tap_t[f"a{l}"][:],
                                        a_next[:].rearrange("p j b -> p (j b)"))
                a_cur = a_next

            # ---- output: feature-major [OUT, BS] (SP stream) ----
            for j in range(OUT // 128):
                nc.sync.dma_start(y_out[j * 128:(j + 1) * 128, :], a_cur[:, j, :])

    nc.compile()
    return nc


def _get_nc():
    if "nc" not in _CACHED:
        _CACHED["nc"] = _build_nc()
    return _CACHED["nc"]


def kernel(**inputs):
    from concourse.bass_utils import run_bass_kernel_spmd

    x = np.asarray(inputs["x"], dtype=np.float32)
    ws = {l: np.ascontiguousarray(np.asarray(inputs[f"w{l}"], dtype=np.float32))
          for l in (1, 2, 3, 4)}
    bs = {l: np.ascontiguousarray(
        np.asarray(inputs[f"b{l}"], dtype=np.float32).reshape(-1, 128).T)
        for l in (1, 2, 3, 4)}

    nc = _get_nc()
    in_maps = []
    for c in range(N_CORES):
        m = {
            "xst": np.ascontiguousarray(x[c * BS:(c + 1) * BS].T),
            "w1s": ws[1][c * HS:(c + 1) * HS],
            "w2s": ws[2][c * HS:(c + 1) * HS],
            "w3s": ws[3][c * HS:(c + 1) * HS],
            "w4s": ws[4][c * OS:(c + 1) * OS],
            "b1": bs[1], "b2": bs[2], "b3": bs[3], "b4": bs[4],
        }
        in_maps.append(m)

    res = run_bass_kernel_spmd(nc, in_maps, list(range(N_CORES)))
    out = np.empty((B, OUT), dtype=np.float32)
    for c in range(N_CORES):
        out[c * BS:(c + 1) * BS] = res.results[c]["y"].T
    return out


if __name__ == "__main__":
    rng = np.random.default_rng(0)
    ins = {
        "x": rng.standard_normal((B, IN)).astype(np.float32),
        "w1": (rng.standard_normal((H, IN)) * 0.1).astype(np.float32),
        "b1": np.zeros(H, np.float32),
        "w2": (rng.standard_normal((H, H)) * 0.1).astype(np.float32),
        "b2": np.zeros(H, np.float32),
        "w3": (rng.standard_normal((H, H)) * 0.1).astype(np.float32),
        "b3": np.zeros(OUT if False else H, np.float32),
        "w4": (rng.standard_normal((OUT, H)) * 0.1).astype(np.float32),
        "b4": np.zeros(OUT, np.float32),
    }
    y = kernel(**ins)
    print("kernel ran, output shape", y.shape, "mean", float(y.mean()))


# revision 17
# speedup vs baseline: 1.7358x; 1.0045x over previous
"""FP4Net (bnb-FP4 quantize-dequantize 4-layer MLP) Trainium2 kernel.

Strategy (8 NeuronCores):
  - Data-parallel over batch for the matmuls: each core handles 1024 of 8192 rows.
  - FP4 quant-dequant of the weights is sharded 8x across cores (by output-row
    blocks, keeping the 64-elem FP4 blocks intact), computed exactly with fp32
    bit tricks on the vector engine, stored transposed (W.T layout) in fp16,
    then AllGathered so every core has all dequantized weights.
  - 4 chained fp16 matmul layers (fp32 PSUM accumulate); bias+ReLU epilogues on
    the vector engine, sigmoid on the scalar engine; activations resident in
    SBUF feature-major.
  - Engine streams are kept separate to avoid sequencer head-of-line blocking:
    SP: bias loads + weight-strip loads + output stores;
    ACT: x staging + all dequant-phase DMAs + final sigmoid;
    DVE: dequant compute + ReLU epilogues (emission-interleaved);
    GpSimd: AllGathers. Dequant of weight l+1 overlaps layer l's matmuls.

Rounding trick: with g = 3*w/scale, the bnb FP4 codebook {0, 1/192, 1/6, 1/4,
1/3, 1/2, 2/3, 1} maps to {0, 1/64, 1/2, 3/4, 1, 3/2, 2, 3}: round-to-nearest
over that set == round g to 1 stored mantissa bit (round-half-up via exact
small-significand integer adds), clamped below at 1/2, plus a two-threshold
step for the {0, 1/64} region. Verified bit-exact vs the jax reference modulo
~1-ulp boundary fuzz (~1 flipped element per 16M weights on the actual data).
"""
import sys
import numpy as np

for _p in ("/opt/trn_rl_repo", "/root/.axon_site/_ro/trn_rl_repo"):
    if _p not in sys.path:
        sys.path.append(_p)

N_CORES = 8
B, IN, H, OUT = 8192, 1024, 4096, 1024
BS = B // N_CORES          # batch shard per core
HS = H // N_CORES          # hidden-row shard per core (w1/w2/w3)
OS = OUT // N_CORES        # out-row shard per core (w4)

# FP4 codebook-derived threshold constants (g-space = 3*norm), f64 precision
_FP4_POS = np.array([0.0, 0.0052083333, 0.6666667, 1.0, 0.3333333, 0.5,
                     0.1666667, 0.25], dtype=np.float32)
_CS = np.sort(_FP4_POS).astype(np.float64)
_TL = float(np.float32(3.0 * (_CS[0] + _CS[1]) / 2.0))
_TH = float(np.float32(3.0 * (_CS[1] + _CS[2]) / 2.0))
LO_BITS = int(np.float32(1.0 / 64).view(np.uint32))   # 0x3C800000
BIG_BITS = 0x40400000                                  # bits of 3.0


def _i32(x):
    return int(np.uint32(x).view(np.int32))


_CACHED = {}

# weight dims per layer: (rows of W == dout, k == contraction)
WDIMS = {1: (H, IN), 2: (H, H), 3: (H, H), 4: (OUT, H)}
FDQ = 512          # dequant chunk free-size (fp32 elems per partition)
NBQ = FDQ // 64    # fp4 blocks per chunk


def _build_nc(taps=False):
    import concourse.bass as bass
    import concourse.mybir as mybir
    import concourse.tile as tile
    from concourse import bacc

    dt = mybir.dt
    Alu = mybir.AluOpType
    Act = mybir.ActivationFunctionType

    nc = bacc.Bacc("TRN2", target_bir_lowering=False, debug=False,
                   num_devices=N_CORES)

    # ---- I/O ----
    xs = nc.dram_tensor("xst", [IN, BS], dt.float32, kind="ExternalInput")
    w_in = {
        1: nc.dram_tensor("w1s", [HS, IN], dt.float32, kind="ExternalInput"),
        2: nc.dram_tensor("w2s", [HS, H], dt.float32, kind="ExternalInput"),
        3: nc.dram_tensor("w3s", [HS, H], dt.float32, kind="ExternalInput"),
        4: nc.dram_tensor("w4s", [OS, H], dt.float32, kind="ExternalInput"),
    }
    b_in = {
        1: nc.dram_tensor("b1", [128, H // 128], dt.float32, kind="ExternalInput"),
        2: nc.dram_tensor("b2", [128, H // 128], dt.float32, kind="ExternalInput"),
        3: nc.dram_tensor("b3", [128, H // 128], dt.float32, kind="ExternalInput"),
        4: nc.dram_tensor("b4", [128, OUT // 128], dt.float32, kind="ExternalInput"),
    }
    y_out = nc.dram_tensor("y", [OUT, BS], dt.float32, kind="ExternalOutput")

    # ---- internal DRAM: dequantized W.T-layout shards + AllGather outputs ----
    dq_shard = {}
    dq_full = {}   # l -> list of gathered half tensors (1 for w4)
    for l, (dout, k) in WDIMS.items():
        rs = dout // N_CORES
        nrt = rs // 128
        dq_shard[l] = nc.dram_tensor(f"dqs{l}", [nrt, k, 128], dt.float16)
        if nrt > 1:
            dq_full[l] = [
                nc.dram_tensor(f"dqf{l}{h}", [N_CORES * nrt // 2, k, 128],
                               dt.float16, addr_space="Shared")
                for h in range(2)]
        else:
            dq_full[l] = [nc.dram_tensor(f"dqf{l}0", [N_CORES * nrt, k, 128],
                                         dt.float16, addr_space="Shared")]

    tap_t = {}
    if taps:
        tap_t["a0"] = nc.dram_tensor("tap_a0", [128, (IN // 128) * BS],
                                     dt.float16, kind="ExternalOutput")
        for l in (1, 2, 3):
            tap_t[f"a{l}"] = nc.dram_tensor(f"tap_a{l}", [128, (H // 128) * BS],
                                            dt.float16, kind="ExternalOutput")

    with tile.TileContext(nc) as tc:
        with (
            tc.tile_pool(name="const", bufs=1) as cpool,
            tc.tile_pool(name="bias", bufs=1) as bpool,
            tc.tile_pool(name="xload", bufs=2) as xpool,
            tc.tile_pool(name="a0", bufs=1) as a0pool,
            tc.tile_pool(name="acts", bufs=2) as apool,
            tc.tile_pool(name="dqin", bufs=4) as dqin_pool,
            tc.tile_pool(name="dqtmp", bufs=1) as dqtmp_pool,
            tc.tile_pool(name="dqout", bufs=3) as dqout_pool,
            tc.tile_pool(name="wt", bufs=6) as wpool,
            tc.tile_pool(name="psum", bufs=8, space="PSUM") as pspool,
        ):
            # int32 constants for scalar_tensor_tensor scalars
            c_lo = cpool.tile([128, 1], dt.int32)
            nc.vector.memset(c_lo[:], _i32(LO_BITS))
            c_half = cpool.tile([128, 1], dt.int32)
            nc.vector.memset(c_half[:], _i32(0x3F000000))
            c_sign = cpool.tile([128, 1], dt.int32)
            nc.vector.memset(c_sign[:], _i32(0x80000000))

            # ---- biases -> SBUF [128, ntiles] (SP stream, startup) ----
            b_sb = {}
            for l, (dout, _k) in WDIMS.items():
                nj = dout // 128
                bt = bpool.tile([128, nj], dt.float32, tag=f"bias{l}")
                nc.sync.dma_start(bt[:], b_in[l][:])
                b_sb[l] = bt

            def emit_dq_tile(l, r, cix):
                """One dequant chunk: [128 h-rows, FDQ k] of weight l's shard."""
                w = dqin_pool.tile([128, NBQ, 64], dt.float32, tag="dqw")
                nc.scalar.dma_start(
                    w[:],
                    w_in[l][r * 128:(r + 1) * 128, cix * FDQ:(cix + 1) * FDQ]
                    .rearrange("p (b i) -> p b i", i=64))
                scale = dqtmp_pool.tile([128, NBQ, 1], dt.float32, tag="scale")
                nc.vector.tensor_reduce(scale[:], w[:], axis=mybir.AxisListType.X,
                                        op=Alu.max, apply_absolute_value=True)
                recip = dqtmp_pool.tile([128, NBQ, 1], dt.float32, tag="recip")
                nc.vector.reciprocal(recip[:], scale[:])
                s3 = dqtmp_pool.tile([128, NBQ, 1], dt.float32, tag="s3")
                nc.vector.tensor_scalar_mul(s3[:], scale[:], 1.0 / 3.0)
                g = dqtmp_pool.tile([128, NBQ, 64], dt.float32, tag="g")
                nc.vector.scalar_tensor_tensor(
                    g[:], w[:], 3.0, recip[:].broadcast_to((128, NBQ, 64)),
                    op0=Alu.mult, op1=Alu.mult)
                gi = g[:].bitcast(dt.int32)
                # NOTE: DVE ops must never write in-place onto their own input
                # (dual-port perf modes race), and int adds must keep few
                # significant bits (the int ALU path is fp32-internal).
                ta = dqtmp_pool.tile([128, NBQ, 64], dt.int32, tag="ta")
                nc.vector.tensor_scalar(ta[:], gi, _i32(0x7FFFFFFF), None,
                                        op0=Alu.bitwise_and)  # m0 = |g| bits
                tb = dqtmp_pool.tile([128, NBQ, 64], dt.int32, tag="tb")
                nc.vector.tensor_scalar(tb[:], ta[:], _i32(0xFFC00000), None,
                                        op0=Alu.bitwise_and)  # trunc
                tc_ = dqtmp_pool.tile([128, NBQ, 64], dt.int32, tag="tc")
                nc.vector.tensor_scalar(tc_[:], ta[:], _i32(0x00200000), _i32(1),
                                        op0=Alu.bitwise_and,
                                        op1=Alu.logical_shift_left)  # half-bit<<1
                te = dqtmp_pool.tile([128, NBQ, 64], dt.int32, tag="te")
                nc.vector.tensor_tensor(te[:], tb[:], tc_[:],
                                        op=Alu.add)  # r2a (exact: 10+1 sig bits)
                af = ta[:].bitcast(dt.float32)  # |g| as float
                # M1L = (|g|>TL)*LO_BITS, M2B = (|g|>TH)*BIG_BITS -- the float
                # products are exact (consts have <=5 significant bits)
                nc.vector.tensor_scalar(tb[:], af, _TL, float(LO_BITS),
                                        op0=Alu.is_gt, op1=Alu.mult)
                td = dqtmp_pool.tile([128, NBQ, 64], dt.int32, tag="td")
                nc.vector.tensor_scalar(td[:], af, _TH, float(BIG_BITS),
                                        op0=Alu.is_gt, op1=Alu.mult)
                nc.vector.tensor_tensor(tc_[:], tb[:], td[:],
                                        op=Alu.add)  # sel (disjoint bits)
                nc.vector.scalar_tensor_tensor(
                    tb[:], te[:], c_half[:], tc_[:],
                    op0=Alu.max, op1=Alu.min)  # mag
                nc.vector.scalar_tensor_tensor(
                    ta[:], gi, c_sign[:], tb[:],
                    op0=Alu.bitwise_and, op1=Alu.bitwise_or)  # signed
                dq = dqout_pool.tile([128, NBQ, 64], dt.float16, tag="dq")
                nc.vector.tensor_tensor(
                    dq[:], ta[:].bitcast(dt.float32),
                    s3[:].broadcast_to((128, NBQ, 64)), op=Alu.mult)
                # transpose-store into W.T layout shard
                dqt = dqout_pool.tile([128, FDQ // 128, 128], dt.float16,
                                      tag="dqt")
                nc.sync.dma_start_transpose(
                    dqt[:], dq[:].rearrange("p b i -> p (b i)"))
                nc.gpsimd.dma_start(
                    dq_shard[l][r, cix * FDQ:(cix + 1) * FDQ, :]
                    .rearrange("(c p) h -> p c h", p=128),
                    dqt[:])

            def dq_tiles_of(l):
                rs = WDIMS[l][0] // N_CORES
                for r in range(rs // 128):
                    for cix in range(WDIMS[l][1] // FDQ):
                        yield (l, r, cix)

            def emit_allgather_half(l, h):
                nrt = WDIMS[l][0] // N_CORES // 128
                if nrt > 1:
                    ins = dq_shard[l][h * (nrt // 2):(h + 1) * (nrt // 2)]
                else:
                    ins = dq_shard[l][:]
                nc.gpsimd.collective_compute(
                    "AllGather", Alu.bypass,
                    replica_groups=[list(range(N_CORES))],
                    ins=[ins],
                    outs=[dq_full[l][h][:]],
                )

            def dq_emitter(l):
                """Generator: send n -> emits next n dq tiles of weight l,
                issuing each half's AllGather as soon as its tiles are done."""
                tiles = list(dq_tiles_of(l))
                nhalf = len(dq_full[l])
                per_half = len(tiles) // nhalf
                done = 0
                while done < len(tiles):
                    n = yield
                    for _ in range(n or 1):
                        if done >= len(tiles):
                            break
                        emit_dq_tile(*tiles[done])
                        done += 1
                        if done % per_half == 0:
                            emit_allgather_half(l, done // per_half - 1)
                while True:
                    yield

            # dequant w1 up front (both halves + AGs)
            em1 = dq_emitter(1)
            next(em1)
            em1.send(len(list(dq_tiles_of(1))))

            # ---- x (host-pre-transposed) load + cast fp16 -> A0 ----
            a_cur = a0pool.tile([128, IN // 128, BS], dt.float16)
            for jk in range(IN // 128):
                xt = xpool.tile([128, BS], dt.float32, tag="xt")
                nc.scalar.dma_start(xt[:], xs[jk * 128:(jk + 1) * 128, :])
                nc.vector.tensor_copy(a_cur[:, jk, :], xt[:])

            if taps:
                nc.scalar.dma_start(tap_t["a0"][:],
                                    a_cur[:].rearrange("p j b -> p (j b)"))

            # ---- matmul layers; layer l interleaves dequant of weight l+1 ----
            for l, (dout, K) in WDIMS.items():
                nj = dout // 128
                nk = K // 128
                nhalf = len(dq_full[l])
                out_dt = dt.float32 if l == 4 else dt.float16
                a_next = apool.tile([128, nj, BS], out_dt, tag="acts")
                emitter = None
                if l < 4:
                    emitter = dq_emitter(l + 1)
                    next(emitter)
                half = nk // 2
                # consume j in AG-half order: all first-half tiles, then second
                if nhalf == 2:
                    j_order = [c * 4 + hh * 2 + r
                               for hh in range(2) for c in range(N_CORES)
                               for r in range(2)]
                else:
                    j_order = list(range(nj))
                for j in j_order:
                    if nhalf == 2:
                        hsel, lt = (0, (j // 4) * 2 + j % 4) if j % 4 < 2 \
                            else (1, (j // 4) * 2 + j % 4 - 2)
                    else:
                        hsel, lt = 0, j
                    src_t = dq_full[l][hsel]
                    wts = []
                    for i0 in (0, half):
                        wt_h = wpool.tile([128, half, 128], dt.float16, tag="wt")
                        nc.sync.dma_start(
                            wt_h[:],
                            src_t[lt, i0 * 128:(i0 + half) * 128, :]
                            .rearrange("(i p) h -> p i h", p=128))
                        wts.append(wt_h)
                    ps = []
                    for _n in range(BS // 512):
                        ps_t = pspool.tile([128, 512], dt.float32, tag="ps")
                        ps.append(ps_t)
                    for i in range(nk):
                        for n in range(BS // 512):
                            nc.tensor.matmul(
                                ps[n][:], wts[i // half][:, i % half, :],
                                a_cur[:, i, n * 512:(n + 1) * 512],
                                start=(i == 0), stop=(i == nk - 1))
                    for n in range(BS // 512):
                        if l == 4:
                            nc.scalar.activation(
                                a_next[:, j, n * 512:(n + 1) * 512], ps[n][:],
                                Act.Sigmoid, bias=b_sb[l][:, j:j + 1], scale=1.0)
                        else:
                            nc.scalar.activation(
                                a_next[:, j, n * 512:(n + 1) * 512], ps[n][:],
                                Act.Relu, bias=b_sb[l][:, j:j + 1], scale=1.0)
                    # interleave next weight's dequant (front-loaded, 2 per j)
                    if emitter is not None:
                        emitter.send(2)
                if taps and l < 4:
                    nc.scalar.dma_start(tap_t[f"a{l}"][:],
                                        a_next[:].rearrange("p j b -> p (j b)"))
                a_cur = a_next

            # ---- output: feature-major [OUT, BS] (SP stream) ----
            for j in range(OUT // 128):
                nc.sync.dma_start(y_out[j * 128:(j + 1) * 128, :], a_cur[:, j, :])

    nc.compile()
    return nc


def _get_nc():
    if "nc" not in _CACHED:
        _CACHED["nc"] = _build_nc()
    return _CACHED["nc"]


def kernel(**inputs):
    from concourse.bass_utils import run_bass_kernel_spmd

    x = np.asarray(inputs["x"], dtype=np.float32)
    ws = {l: np.ascontiguousarray(np.asarray(inputs[f"w{l}"], dtype=np.float32))
          for l in (1, 2, 3, 4)}
    bs = {l: np.ascontiguousarray(
        np.asarray(inputs[f"b{l}"], dtype=np.float32).reshape(-1, 128).T)
        for l in (1, 2, 3, 4)}

    nc = _get_nc()
    in_maps = []
    for c in range(N_CORES):
        m = {
            "xst": np.ascontiguousarray(x[c * BS:(c + 1) * BS].T),
            "w1s": ws[1][c * HS:(c + 1) * HS],
            "w2s": ws[2][c * HS:(c + 1) * HS],
            "w3s": ws[3][c * HS:(c + 1) * HS],
            "w4s": ws[4][c * OS:(c + 1) * OS],
            "b1": bs[1], "b2": bs[2], "b3": bs[3], "b4": bs[4],
        }
        in_maps.append(m)

    res = run_bass_kernel_spmd(nc, in_maps, list(range(N_CORES)))
    out = np.empty((B, OUT), dtype=np.float32)
    for c in range(N_CORES):
        out[c * BS:(c + 1) * BS] = res.results[c]["y"].T
    return out


if __name__ == "__main__":
    rng = np.random.default_rng(0)
    ins = {
        "x": rng.standard_normal((B, IN)).astype(np.float32),
        "w1": (rng.standard_normal((H, IN)) * 0.1).astype(np.float32),
        "b1": np.zeros(H, np.float32),
        "w2": (rng.standard_normal((H, H)) * 0.1).astype(np.float32),
        "b2": np.zeros(H, np.float32),
        "w3": (rng.standard_normal((H, H)) * 0.1).astype(np.float32),
        "b3": np.zeros(OUT if False else H, np.float32),
        "w4": (rng.standard_normal((OUT, H)) * 0.1).astype(np.float32),
        "b4": np.zeros(OUT, np.float32),
    }
    y = kernel(**ins)
    print("kernel ran, output shape", y.shape, "mean", float(y.mean()))
